# revision 25
# baseline (speedup 1.0000x reference)
"""Llama GQA attention layer (prefill with KV cache) as a Trainium2 Bass/Tile
kernel, tensor-parallel over heads across 8 NeuronCores.

Contract: kernel(**inputs) takes the FULL unsharded inputs (numpy, fp32) and
returns the FULL [B, S, H] output. Sharding: each core gets 4 q-heads and the
matching kv-head (w_qkv column shard, w_o row shard); hidden_states is
replicated (fed pre-transposed); the o_proj row-parallel all-reduce is a host
numpy sum over the 8 partial outputs.

v3 changes over v2:
- QKV projection on 512-token blocks (N=512 moving) -> half the matmul count.
- Stage 0 is k-outer across 6 concurrent PSUM banks, fed by a k-major DMA
  stream, so the first matmul starts ~2us in and is never DMA-starved.
- Causal triangle trimming: boundary key-chunks only compute the visible
  query subrange; the mask shrinks to one [128,128] lower-triangle tile.
- o_proj iterates hb-outer with rolling [128, 4*512] w_o tiles; y is written
  per (128-token, 1024-col) pair chunk.
- DMA-instruction count trimmed (each HWDGE descriptor fetch serializes for
  ~625ns); rope half-swaps and w_o loads ride the Pool engine's SWDGE path.
- Separate PSUM rings for qkv groups vs o_proj groups; softmax-sum tiles
  share the scores ring; fractional filler cadence avoids end-of-stage
  starvation.

Self-contained: hardcodes all shapes; only imports the toolchain from
/opt/trn_rl_repo.
"""

import sys

if "/opt/trn_rl_repo" not in sys.path:
    sys.path.insert(0, "/opt/trn_rl_repo")

import numpy as np

import concourse.bass as bass
import concourse.mybir as mybir
import concourse.tile as tile
from concourse import bacc
from concourse.bass_utils import run_bass_kernel_spmd
from concourse.masks import make_identity

# Problem shapes
B, S, P = 2, 1024, 1024
T = P + S                      # 2048 total kv positions
H, NQ, NKV, D = 4096, 32, 8, 128
G = NQ // NKV                  # 4 q heads per kv head
NCORES = 8
GPC = NQ // NCORES             # 4 q heads per core
SCALE = 1.0 / float(np.sqrt(D))

BS = B * S                     # 2048 tokens (b-major)
QKV_COLS = GPC * D + 2 * D     # 768 per-core qkv output columns
KCH = 32                       # H // 128 contraction chunks
MCH = QKV_COLS // 128          # 6 output chunks (0-3 q, 4 k, 5 v)
HN = 4                         # 512-token blocks in QKV projection
HNW = BS // HN                 # 512
HBW = 512                      # o_proj output-column chunk
NHB = H // HBW                 # 8 hb chunks
F16 = mybir.dt.float16
F32 = mybir.dt.float32


def _build_program():
    nc = bacc.Bacc("TRN2", target_bir_lowering=False, debug=False,
                   num_devices=NCORES)

    xT = nc.dram_tensor("xT", [H, BS], F16, kind="ExternalInput").ap()
    # wqkv k-major: [128, k*MCH*128 + m*128 + c]
    wqkv = nc.dram_tensor("wqkv", [128, KCH * MCH * 128], F16,
                          kind="ExternalInput").ap()
    # wo hb-major: [128, hb*GPC*512 + g*512 + c]
    wo = nc.dram_tensor("wo", [128, NHB * GPC * HBW], F16,
                        kind="ExternalInput").ap()
    cosT_d = nc.dram_tensor("cosT", [128, S], F16, kind="ExternalInput").ap()
    ssinT_d = nc.dram_tensor("ssinT", [128, S], F16, kind="ExternalInput").ap()
    kcT_d = nc.dram_tensor("kcT", [128, B * P], F16, kind="ExternalInput").ap()
    vc_d = nc.dram_tensor("vc", [B * P, D], F16, kind="ExternalInput").ap()
    mask_d = nc.dram_tensor("masks", [128, 128], F16,
                            kind="ExternalInput").ap()
    y = nc.dram_tensor("y", [BS, H], F16, kind="ExternalOutput").ap()

    with tile.TileContext(nc) as tc:
        with (tc.tile_pool(name="persist", bufs=1) as pp,
              tc.tile_pool(name="xt", bufs=2) as xtp,
              tc.tile_pool(name="rope", bufs=2) as ropep,
              tc.tile_pool(name="vt", bufs=1) as vtp,
              tc.tile_pool(name="pt", bufs=3) as ptp,
              tc.tile_pool(name="accs", bufs=2) as accp,
              tc.tile_pool(name="rc", bufs=1) as rcp,
              tc.tile_pool(name="bc", bufs=2) as bcp,
              tc.tile_pool(name="ys", bufs=3) as ysp,
              tc.tile_pool(name="wop", bufs=5) as wop):
            # Persistent SBUF tensors. Layouts (all [128 partitions, free]):
            #  qT: head-dim on partitions, cols g*2048 + b*1024 + s
            #  kT: cols b*2048 + t  (t<1024 cache, t>=1024 new)
            #  v_sb: [t, d] chunks; chunk (b, tc) at col 128*(16b+tc),
            #        tc 0-7 cache, 8-15 new
            #  outT_sb: cols b*4096 + g*1024 + s
            wq_sb = pp.tile([128, KCH * MCH * 128], F16, tag="wq_sb")
            qT = pp.tile([128, GPC * BS], F16, tag="qT")
            kT = pp.tile([128, B * T], F16, tag="kT")
            v_sb = pp.tile([128, B * T], F16, tag="v_sb")
            cosT = pp.tile([128, S], F16, tag="cosT")
            ssinT = pp.tile([128, S], F16, tag="ssinT")
            mask_sb = pp.tile([128, 128], F16, tag="masks")
            outT_sb = pp.tile([128, B * GPC * S], F16, tag="outT_sb")
            ident = pp.tile([128, 128], F16, tag="ident")
            ones = pp.tile([128, 1], F16, tag="ones")

            nc.vector.memset(ones[:], 1.0)
            make_identity(nc, ident[:])

            xT_r = xT.rearrange("(k p) t -> p k t", p=128)
            xt_tiles = {}

            def xt_alloc(hn):
                xt_t = xtp.tile([128, KCH * HNW], F16, tag="xt",
                                name=f"xt{hn}")
                xt_tiles[hn] = xt_t
                return xt_t[:].rearrange("p (k t) -> p k t", k=KCH)

            def xt_load(hn, split=4):
                t0 = hn * HNW
                dst = xt_alloc(hn)
                ksz = KCH // split
                for i in range(split):
                    nc.sync.dma_start(
                        dst[:, i * ksz:(i + 1) * ksz, :],
                        xT_r[:, i * ksz:(i + 1) * ksz, t0:t0 + HNW])

            # ---- DMA issue (priority order) ----
            # Stage-0 stream, k-major. First 4 k chunks individually (fast
            # first-matmul), the rest in pairs (fewer HWDGE descriptor
            # fetches). Supply ~2 k per 1.3us vs consumption 2 per 2.6us.
            xt0_dst = xt_alloc(0)
            kk = 0
            while kk < KCH:
                step = 1 if kk < 4 else 2
                nc.sync.dma_start(xt0_dst[:, kk:kk + step, :],
                                  xT_r[:, kk:kk + step, 0:HNW])
                c0 = kk * MCH * 128
                if kk == 0:
                    # m=0 chunk alone so the first matmul starts sooner
                    nc.sync.dma_start(wq_sb[:, 0:128], wqkv[:, 0:128])
                    nc.sync.dma_start(wq_sb[:, 128:MCH * 128],
                                      wqkv[:, 128:MCH * 128])
                else:
                    nc.sync.dma_start(wq_sb[:, c0:c0 + step * MCH * 128],
                                      wqkv[:, c0:c0 + step * MCH * 128])
                kk += step
                if kk == 16:
                    nc.sync.dma_start(cosT[:], cosT_d[:])
                    nc.sync.dma_start(ssinT[:], ssinT_d[:])
                    nc.sync.dma_start(mask_sb[:], mask_d[:])
            # KV cache (b0), then xt1, then KV cache (b1): matches the order
            # stages 1-2 consume them.
            vc_r = vc_d.rearrange("(b tc p) d -> p b tc d", b=B, p=128)

            def kv_load(b):
                nc.sync.dma_start(kT[:, b * T:b * T + P],
                                  kcT_d[:, b * P:(b + 1) * P])
                nc.sync.dma_start(
                    v_sb[:, b * T:b * T + P].rearrange(
                        "p (tc d) -> p tc d", tc=8),
                    vc_r[:, b])

            kv_load(0)
            xt_load(1)

            # ---- emitters -------------------------------------------------
            def rope_chunk(src_ap, c0, s0, swdge):
                """RoPE over one 512-wide chunk at col c0, in place: 2
                half-swap DMAs, then rot *= ssin, src *= cos, src += rot
                (all DVE, no scratch beyond rot)."""
                rot = ropep.tile([128, 512], F16, tag="rot", name="rot")
                eng = nc.gpsimd if swdge else nc.sync
                eng.dma_start(rot[0:64, :], src_ap[64:128, c0:c0 + 512])
                eng.dma_start(rot[64:128, :], src_ap[0:64, c0:c0 + 512])
                nc.vector.tensor_mul(rot[:], rot[:], ssinT[:, s0:s0 + 512])
                nc.vector.tensor_mul(src_ap[:, c0:c0 + 512],
                                     src_ap[:, c0:c0 + 512],
                                     cosT[:, s0:s0 + 512])
                nc.vector.tensor_add(src_ap[:, c0:c0 + 512],
                                     src_ap[:, c0:c0 + 512], rot[:])

            def evac_m(hn, b, s0, m, ps, trp):
                """Evacuate one [128,512] qkv psum group; rope q/k; for v,
                transpose into v_sb (yields after each transpose matmul)."""
                ev_dve = (m % 2 == 1)
                swdge = hn != 0
                if m < GPC:
                    dst = qT[:, m * BS + b * S + s0:
                                m * BS + b * S + s0 + HNW]
                    if ev_dve:
                        nc.vector.tensor_copy(dst, ps[:])
                    else:
                        nc.scalar.copy(dst, ps[:])
                    rope_chunk(qT, m * BS + b * S + s0, s0, swdge)
                elif m == GPC:
                    dst = kT[:, b * T + P + s0:b * T + P + s0 + HNW]
                    nc.scalar.copy(dst, ps[:])
                    rope_chunk(kT, b * T + P + s0, s0, swdge)
                else:
                    vt = vtp.tile([128, HNW], F16, tag="vt", name=f"vt{hn}")
                    nc.vector.tensor_copy(vt[:], ps[:])
                    tr = trp.tile([128, HNW], F16, tag="big" if trp is not None and trp.name == "psbig" else "tr", name=f"tr{hn}")
                    for i in range(HNW // 128):
                        nc.tensor.transpose(
                            tr[:, 128 * i:128 * (i + 1)],
                            vt[:, 128 * i:128 * (i + 1)], ident[:])
                        yield
                    vch0 = 16 * b + 8 + s0 // 128
                    nc.vector.tensor_copy(
                        v_sb[:, 128 * vch0:128 * vch0 + HNW], tr[:])

            def qkv_hn(hn):
                """QKV projection for one 512-token block; m-outer, yields
                after each matmul so attention chunks can interleave."""
                b = hn // (HN // B)
                s0 = (hn % (HN // B)) * HNW   # within-batch token offset
                xt_t = xt_tiles[hn]
                for m in range(MCH):
                    ps = psbig.tile([128, 512], F32, tag="big",
                                    name=f"qkv{hn}_{m}")
                    for k in range(KCH):
                        c0 = (k * MCH + m) * 128
                        nc.tensor.matmul(
                            ps[:], wq_sb[:, c0:c0 + 128],
                            xt_t[:, k * HNW:(k + 1) * HNW],
                            start=(k == 0), stop=(k == KCH - 1))
                        yield
                    yield from evac_m(hn, b, s0, m, ps, psbig)

            def qkv_hn0_kmajor(ps0):
                """Stage-0 QKV for block 0: k-outer over 6 concurrent PSUM
                groups while the DMA stream is the limiter (k < KT), then
                finish each group m-sequentially so evacs+ropes stagger."""
                KT = KCH - 4
                xt_t = xt_tiles[0]
                groups = [ps0.tile([128, 512], F32, tag=f"q{m}",
                                   name=f"qkv0_{m}") for m in range(MCH)]
                for k in range(KT):
                    for m in range(MCH):
                        c0 = (k * MCH + m) * 128
                        nc.tensor.matmul(
                            groups[m][:], wq_sb[:, c0:c0 + 128],
                            xt_t[:, k * HNW:(k + 1) * HNW],
                            start=(k == 0), stop=False)
                for m in (GPC, 0, 1, 2, 3, MCH - 1):
                    for k in range(KT, KCH):
                        c0 = (k * MCH + m) * 128
                        nc.tensor.matmul(
                            groups[m][:], wq_sb[:, c0:c0 + 128],
                            xt_t[:, k * HNW:(k + 1) * HNW],
                            start=False, stop=(k == KCH - 1))
                    for _ in evac_m(0, 0, 0, m, groups[m], ps0):
                        pass

            def load_wo(hb):
                wo_t = wop.tile([128, GPC * HBW], F16, tag="wo",
                                name=f"wo{hb}")
                c0 = hb * GPC * HBW
                nc.sync.dma_start(wo_t[:], wo[:, c0:c0 + GPC * HBW])
                return wo_t

            def oproj_tail(sc_list, preloaded):
                """Tail o_proj pass (b=1): the first 4 groups emit their
                g<3 matmuls up front (they only need earlier finalizes) so
                the PE stays busy through the last block's finalize chain;
                alternates the op/big PSUM rings for depth 4."""
                b = 1
                engs = [nc.scalar, nc.vector]
                wo_tiles = dict(preloaded)

                def ensure(hb):
                    if hb < NHB and hb not in wo_tiles:
                        wo_tiles[hb] = load_wo(hb)

                ensure(0)
                ensure(1)
                units = []
                for hbp in range(NHB // 2):
                    for sc in sc_list:
                        for h2 in range(2):
                            units.append((2 * hbp + h2, sc, hbp, h2))
                NPRO = 4
                pro_ps = []
                for i, (hb, sc, hbp, h2) in enumerate(units[:NPRO]):
                    ops = psbig.tile([128, HBW], F32,
                                     tag="op" if i % 2 == 0 else "big",
                                     name=f"opt{sc}_{hb}")
                    pro_ps.append(ops)
                    for g in range(GPC - 1):
                        lcol = b * GPC * S + g * S + 128 * sc
                        nc.tensor.matmul(
                            ops[:], outT_sb[:, lcol:lcol + 128],
                            wo_tiles[hb][:, g * HBW:(g + 1) * HBW],
                            start=(g == 0), stop=False)
                ys = None
                for i, (hb, sc, hbp, h2) in enumerate(units):
                    if i == 2:
                        ensure(2)
                        ensure(3)
                    elif i == 10:
                        ensure(4)
                        ensure(5)
                    elif i == 18:
                        ensure(6)
                        ensure(7)
                    eng = engs[i % 2]
                    if i < NPRO:
                        ops = pro_ps[i]
                        g0 = GPC - 1
                    else:
                        ops = psbig.tile([128, HBW], F32,
                                         tag="op" if i % 2 == 0 else "big",
                                         name=f"opt{sc}_{hb}")
                        g0 = 0
                    for g in range(g0, GPC):
                        lcol = b * GPC * S + g * S + 128 * sc
                        nc.tensor.matmul(
                            ops[:], outT_sb[:, lcol:lcol + 128],
                            wo_tiles[hb][:, g * HBW:(g + 1) * HBW],
                            start=(g == 0 and g0 == 0),
                            stop=(g == GPC - 1))
                    if h2 == 0:
                        ys = ysp.tile([128, 2 * HBW], F16, tag="ys",
                                      name=f"yst{sc}_{hbp}")
                    dst = ys[:, h2 * HBW:(h2 + 1) * HBW]
                    if eng is nc.scalar:
                        eng.copy(dst, ops[:])
                    else:
                        eng.tensor_copy(dst, ops[:])
                    nc.sync.dma_start(
                        y[b * S + 128 * sc:b * S + 128 * (sc + 1),
                          HBW * hb:HBW * (hb + 1)], dst)

            def oproj_pass(b, sc_list, preloaded=None, single_dma=False):
                """One o_proj pass: hb-pair-outer; per group 4 contraction
                matmuls + evac copy into half a [128,1024] ys tile; one y DMA
                per pair. wo tiles roll through `wop` (SWDGE loads) with a
                one-pair prefetch distance."""
                engs = [nc.scalar, nc.vector]
                i = 0
                wo_tiles = dict(preloaded or {})

                def ensure(hb):
                    if hb < NHB and hb not in wo_tiles:
                        wo_tiles[hb] = load_wo(hb)

                ensure(0)
                ensure(1)
                for hbp in range(NHB // 2):
                    for si, sc in enumerate(sc_list):
                        for h2 in range(2):
                            hb = 2 * hbp + h2
                            eng = engs[i % 2]
                            i += 1

                            def unit(b=b, sc=sc, hb=hb, h2=h2, si=si,
                                     hbp=hbp, eng=eng, wo_t=wo_tiles[hb]):
                                if si == 0 and h2 == 0:
                                    ensure(2 * hbp + 2)
                                    ensure(2 * hbp + 3)
                                ops = psbig.tile([128, HBW], F32, tag="op",
                                                 name=f"op{b}_{sc}_{hb}")
                                for g in range(GPC):
                                    lcol = b * GPC * S + g * S + 128 * sc
                                    nc.tensor.matmul(
                                        ops[:], outT_sb[:, lcol:lcol + 128],
                                        wo_t[:, g * HBW:(g + 1) * HBW],
                                        start=(g == 0), stop=(g == GPC - 1))
                                if h2 == 0:
                                    ys = ysp.tile([128, 2 * HBW], F16,
                                                  tag="ys",
                                                  name=f"ys{b}_{sc}_{hbp}")
                                    oproj_pass.ys = ys
                                ys = oproj_pass.ys
                                dst = ys[:, h2 * HBW:(h2 + 1) * HBW]
                                if eng is nc.scalar:
                                    eng.copy(dst, ops[:])
                                else:
                                    eng.tensor_copy(dst, ops[:])
                                if single_dma:
                                    nc.sync.dma_start(
                                        y[b * S + 128 * sc:
                                          b * S + 128 * (sc + 1),
                                          HBW * hb:HBW * (hb + 1)], dst)
                                elif h2 == 1:
                                    nc.sync.dma_start(
                                        y[b * S + 128 * sc:
                                          b * S + 128 * (sc + 1),
                                          1024 * hbp:1024 * (hbp + 1)],
                                        ys[:])

                            yield unit

            # finalize: normalize one attention block's output.
            # Split in two so PE fillers sit between the sums matmul and
            # the broadcast matmul (which waits on the DVE reciprocal).
            def finalize_a(pend):
                f_acc, f_ot, f_ocol = pend
                sums = psp.tile([128, 512], F32, tag="sc", name="sums")
                nc.tensor.matmul(sums[0:1, :], ones[:], f_acc[:],
                                 start=True, stop=True)
                rc = rcp.tile([1, 512], F16, tag="rc", name="rc")
                with nc.allow_low_precision(reason="softmax denom fits fp16"):
                    nc.vector.reciprocal(rc[:], sums[0:1, :])
                return (rc, f_ot, f_ocol)

            def finalize_b(pend2):
                rc, f_ot, f_ocol = pend2
                bc = bcp.tile([128, 512], F16, tag="bc", name="bc")
                nc.gpsimd.partition_broadcast(bc[:], rc[:])
                nc.vector.tensor_mul(outT_sb[:, f_ocol:f_ocol + 512],
                                     f_ot[:], bc[:])

            pending = [None]

            def attn_block(b, g, j, fillers, cadence):
                """One attention s-block (512 queries): scores+exp+pv over
                n_t kv chunks, pipelined; pulls `cadence` (fractional) filler
                units from `fillers` after each chunk's scores matmul.
                Boundary chunks only compute the visible query subrange."""
                scol = g * BS + b * S + j * 512
                n_t = (P // 128) + 4 * (j + 1)      # causal skip
                acc = accp.tile([128, 512], F16, tag="acc",
                                name=f"acc{b}{g}{j}")
                ot_ps = psp.tile([128, 512], F32, tag="ot",
                                 name=f"ot{b}{g}{j}")
                prev = None
                credit = 0.0
                for ti in range(n_t):
                    if ti < 8:
                        kcol = b * T + 128 * ti
                    else:
                        kcol = b * T + P + 128 * (ti - 8)
                    vch = 16 * b + ti
                    r_idx = (ti - 8) - 4 * j
                    s_lo = 128 * r_idx if (ti >= 8 and 0 <= r_idx < 4) else 0
                    sc_ps = psp.tile([128, 512], F32, tag="sc", name="sc")
                    nc.tensor.matmul(sc_ps[:, s_lo:512],
                                     kT[:, kcol:kcol + 128],
                                     qT[:, scol + s_lo:scol + 512],
                                     start=True, stop=True)
                    pt = ptp.tile([128, 512], F16, tag="pt", name="pt")
                    nc.scalar.activation(pt[:, s_lo:512], sc_ps[:, s_lo:512],
                                         mybir.ActivationFunctionType.Exp,
                                         scale=SCALE)
                    if ti >= 8 and 0 <= r_idx < 4:
                        # diagonal 128-col strip: in-chunk causal triangle
                        nc.vector.tensor_mul(
                            pt[:, s_lo:s_lo + 128], pt[:, s_lo:s_lo + 128],
                            mask_sb[:])
                    if ti == 0:
                        nc.vector.tensor_copy(acc[:], pt[:])
                    else:
                        nc.vector.tensor_add(acc[:, s_lo:512],
                                             pt[:, s_lo:512],
                                             acc[:, s_lo:512])
                    # fillers between the scores and the previous pv;
                    # trimmed chunks leave more PE idle, so weight them up
                    credit += cadence + 3.0 * (s_lo / 512.0)
                    while credit >= 1.0:
                        credit -= 1.0
                        if not next_filler(fillers):
                            break
                    if prev is not None:
                        p_pt, p_vch, p_slo, p_first = prev
                        nc.tensor.matmul(
                            ot_ps[:, p_slo:512],
                            v_sb[:, 128 * p_vch:128 * (p_vch + 1)],
                            p_pt[:, p_slo:512], start=p_first, stop=False)
                    prev = (pt, vch, s_lo, ti == 0)
                    if ti == 0 and pending[0] is not None:
                        attn_block.pend2 = finalize_a(pending[0])
                        pending[0] = None
                    elif ti == 4 and attn_block.pend2 is not None:
                        finalize_b(attn_block.pend2)
                        attn_block.pend2 = None
                p_pt, p_vch, p_slo, p_first = prev
                nc.tensor.matmul(ot_ps[:, p_slo:512],
                                 v_sb[:, 128 * p_vch:128 * (p_vch + 1)],
                                 p_pt[:, p_slo:512], start=p_first, stop=True)
                ocol = b * GPC * S + g * S + j * 512
                pending[0] = (acc, ot_ps, ocol)

            attn_block.pend2 = None

            def next_filler(fillers):
                while fillers:
                    try:
                        u = next(fillers[0])
                        if callable(u):
                            u()
                        return True
                    except StopIteration:
                        fillers.pop(0)
                return False

            def drain(fillers):
                while next_filler(fillers):
                    pass

            # ---- schedule -------------------------------------------------
            # stage 0: qkv(hn0) k-major in its own 6-bank+tr PSUM scope
            with tc.tile_pool(name="ps0", bufs=1, space="PSUM") as ps0:
                qkv_hn0_kmajor(ps0)
            with (tc.tile_pool(name="ps", bufs=2, space="PSUM") as psp,
                  tc.tile_pool(name="psbig", bufs=2, space="PSUM") as psbig):
                # stage 1: attn(b0, j=0) + qkv(hn1). Pre-pull ~36 units so
                # the PE has work while hn0's q/k rope chains complete.
                fill = [qkv_hn(1)]
                for _ in range(32):
                    next_filler(fill)
                for g in range(GPC):
                    attn_block(0, g, 0, fill, cadence=3.2)
                    if g == 1:
                        xt_load(2, split=8)
                drain(fill)
                # stage 2: attn(b0, j=1) + qkv(hn2)
                fill = [qkv_hn(2)]
                for g in range(GPC):
                    attn_block(0, g, 1, fill, cadence=2.85)
                    if g == 1:
                        xt_load(3, split=8)
                    elif g == 2:
                        kv_load(1)
                drain(fill)
                # stage 3: attn(b1, j=0) + qkv(hn3) + o_proj pass A (b0)
                gen_qkv3 = qkv_hn(3)
                genA = oproj_pass(0, range(8))
                fill = [gen_qkv3, genA]
                for g in range(GPC):
                    attn_block(1, g, 0, fill, cadence=4.1)
                drain([gen_qkv3])
                # stage 4: attn(b1, j=1) + o_proj passes A remainder + B.
                # Prime pass B so its first wo tile loads ahead of use.
                genB = oproj_pass(1, range(0, 4))
                uB0 = next(genB)

                def chainB():
                    yield uB0
                    yield from genB

                fill = [genA, chainB()]
                woC = {}
                for g in range(GPC):
                    attn_block(1, g, 1, fill, cadence=1.2)
                    if g == 1:
                        woC = {0: load_wo(0), 1: load_wo(1)}
                # normalize the last block while leftover fillers keep PE
                # busy, then the b1/j1-dependent o_proj tail
                p2 = finalize_a(pending[0])
                pending[0] = None
                drain(fill)
                finalize_b(p2)
                oproj_tail(range(4, 8), woC)

    nc.compile()
    return nc


_PROGRAM = None


def _get_program():
    global _PROGRAM
    if _PROGRAM is None:
        _PROGRAM = _build_program()
    return _PROGRAM


def _shard_inputs(hidden_states, w_qkv, w_o, cos, sin, k_cache, v_cache):
    """Build the 8 per-core input maps (numpy, fp16)."""
    hs = np.asarray(hidden_states, np.float32)
    w_qkv = np.asarray(w_qkv, np.float32)
    w_o = np.asarray(w_o, np.float32)
    cos = np.asarray(cos, np.float32)
    sin = np.asarray(sin, np.float32)
    k_cache = np.asarray(k_cache, np.float32)
    v_cache = np.asarray(v_cache, np.float32)

    xT = np.ascontiguousarray(hs.reshape(BS, H).T.astype(np.float16))
    cosT = np.ascontiguousarray(cos.T.astype(np.float16))
    ssinT = sin.T.astype(np.float16).copy()
    ssinT[0:64] *= -1.0
    ssinT = np.ascontiguousarray(ssinT)

    # lower-triangle tile: mask[t, s] = (s >= t)
    tl = np.arange(128)[:, None]
    sl = np.arange(128)[None, :]
    mask = np.ascontiguousarray((sl >= tl).astype(np.float16))

    in_maps = []
    for c in range(NCORES):
        wq_c = w_qkv[:, c * GPC * D:(c + 1) * GPC * D]
        wk_c = w_qkv[:, NQ * D + c * D:NQ * D + (c + 1) * D]
        wv_c = w_qkv[:, (NQ + NKV) * D + c * D:(NQ + NKV) * D + (c + 1) * D]
        wc = np.concatenate([wq_c, wk_c, wv_c], axis=1)      # [H, 768]
        # k-major: [p, k*MCH*128 + m*128 + col]
        wqkv_r = np.ascontiguousarray(
            wc.reshape(KCH, 128, MCH, 128).transpose(1, 0, 2, 3)
            .reshape(128, KCH * MCH * 128).astype(np.float16))
        wo_c = w_o[c * GPC * D:(c + 1) * GPC * D, :]          # [512, H]
        # hb-major: [p, hb*GPC*512 + g*512 + col]
        wo_r = np.ascontiguousarray(
            wo_c.reshape(GPC, 128, NHB, HBW).transpose(1, 2, 0, 3)
            .reshape(128, NHB * GPC * HBW).astype(np.float16))
        kcT = np.ascontiguousarray(
            k_cache[:, :, c, :].reshape(B * P, D).T.astype(np.float16))
        vc = np.ascontiguousarray(
            v_cache[:, :, c, :].reshape(B * P, D).astype(np.float16))
        in_maps.append(dict(xT=xT, wqkv=wqkv_r, wo=wo_r, cosT=cosT,
                            ssinT=ssinT, kcT=kcT, vc=vc, masks=mask))
    return in_maps


def _run(in_maps, trace=False):
    nc = _get_program()
    return run_bass_kernel_spmd(nc, in_maps, list(range(NCORES)), trace=trace)


def kernel(hidden_states, w_qkv, w_o, cos, sin, k_cache, v_cache):
    in_maps = _shard_inputs(hidden_states, w_qkv, w_o, cos, sin,
                            k_cache, v_cache)
    res = _run(in_maps)
    acc = np.zeros((BS, H), np.float64)
    for c in range(NCORES):
        acc += res.results[c]["y"]
    return acc.astype(np.float32).reshape(B, S, H)


# revision 26
# speedup vs baseline: 1.0078x; 1.0078x over previous
"""Llama GQA attention layer (prefill with KV cache) as a Trainium2 Bass/Tile
kernel, tensor-parallel over heads across 8 NeuronCores.

Contract: kernel(**inputs) takes the FULL unsharded inputs (numpy, fp32) and
returns the FULL [B, S, H] output. Sharding: each core gets 4 q-heads and the
matching kv-head (w_qkv column shard, w_o row shard); hidden_states is
replicated (fed pre-transposed); the o_proj row-parallel all-reduce is a host
numpy sum over the 8 partial outputs.

v3 changes over v2:
- QKV projection on 512-token blocks (N=512 moving) -> half the matmul count.
- Stage 0 is k-outer across 6 concurrent PSUM banks, fed by a k-major DMA
  stream, so the first matmul starts ~2us in and is never DMA-starved.
- Causal triangle trimming: boundary key-chunks only compute the visible
  query subrange; the mask shrinks to one [128,128] lower-triangle tile.
- o_proj iterates hb-outer with rolling [128, 4*512] w_o tiles; y is written
  per (128-token, 1024-col) pair chunk.
- DMA-instruction count trimmed (each HWDGE descriptor fetch serializes for
  ~625ns); rope half-swaps and w_o loads ride the Pool engine's SWDGE path.
- Separate PSUM rings for qkv groups vs o_proj groups; softmax-sum tiles
  share the scores ring; fractional filler cadence avoids end-of-stage
  starvation.

Self-contained: hardcodes all shapes; only imports the toolchain from
/opt/trn_rl_repo.
"""

import sys

if "/opt/trn_rl_repo" not in sys.path:
    sys.path.insert(0, "/opt/trn_rl_repo")

import numpy as np

import concourse.bass as bass
import concourse.mybir as mybir
import concourse.tile as tile
from concourse import bacc
from concourse.bass_utils import run_bass_kernel_spmd
from concourse.masks import make_identity

# Problem shapes
B, S, P = 2, 1024, 1024
T = P + S                      # 2048 total kv positions
H, NQ, NKV, D = 4096, 32, 8, 128
G = NQ // NKV                  # 4 q heads per kv head
NCORES = 8
GPC = NQ // NCORES             # 4 q heads per core
SCALE = 1.0 / float(np.sqrt(D))

BS = B * S                     # 2048 tokens (b-major)
QKV_COLS = GPC * D + 2 * D     # 768 per-core qkv output columns
KCH = 32                       # H // 128 contraction chunks
MCH = QKV_COLS // 128          # 6 output chunks (0-3 q, 4 k, 5 v)
HN = 4                         # 512-token blocks in QKV projection
HNW = BS // HN                 # 512
HBW = 512                      # o_proj output-column chunk
NHB = H // HBW                 # 8 hb chunks
F16 = mybir.dt.float16
F32 = mybir.dt.float32


def _build_program():
    nc = bacc.Bacc("TRN2", target_bir_lowering=False, debug=False,
                   num_devices=NCORES)

    xT = nc.dram_tensor("xT", [H, BS], F16, kind="ExternalInput").ap()
    # wqkv k-major: [128, k*MCH*128 + m*128 + c]
    wqkv = nc.dram_tensor("wqkv", [128, KCH * MCH * 128], F16,
                          kind="ExternalInput").ap()
    # wo hb-major: [128, hb*GPC*512 + g*512 + c]
    wo = nc.dram_tensor("wo", [128, NHB * GPC * HBW], F16,
                        kind="ExternalInput").ap()
    cosT_d = nc.dram_tensor("cosT", [128, S], F16, kind="ExternalInput").ap()
    ssinT_d = nc.dram_tensor("ssinT", [128, S], F16, kind="ExternalInput").ap()
    kcT_d = nc.dram_tensor("kcT", [128, B * P], F16, kind="ExternalInput").ap()
    vc_d = nc.dram_tensor("vc", [B * P, D], F16, kind="ExternalInput").ap()
    mask_d = nc.dram_tensor("masks", [128, 128], F16,
                            kind="ExternalInput").ap()
    y = nc.dram_tensor("y", [BS, H], F16, kind="ExternalOutput").ap()

    with tile.TileContext(nc) as tc:
        with (tc.tile_pool(name="persist", bufs=1) as pp,
              tc.tile_pool(name="xt", bufs=2) as xtp,
              tc.tile_pool(name="rope", bufs=2) as ropep,
              tc.tile_pool(name="vt", bufs=1) as vtp,
              tc.tile_pool(name="pt", bufs=3) as ptp,
              tc.tile_pool(name="accs", bufs=2) as accp,
              tc.tile_pool(name="rc", bufs=1) as rcp,
              tc.tile_pool(name="bc", bufs=2) as bcp,
              tc.tile_pool(name="ys", bufs=3) as ysp,
              tc.tile_pool(name="wop", bufs=5) as wop):
            # Persistent SBUF tensors. Layouts (all [128 partitions, free]):
            #  qT: head-dim on partitions, cols g*2048 + b*1024 + s
            #  kT: cols b*2048 + t  (t<1024 cache, t>=1024 new)
            #  v_sb: [t, d] chunks; chunk (b, tc) at col 128*(16b+tc),
            #        tc 0-7 cache, 8-15 new
            #  outT_sb: cols b*4096 + g*1024 + s
            wq_sb = pp.tile([128, KCH * MCH * 128], F16, tag="wq_sb")
            qT = pp.tile([128, GPC * BS], F16, tag="qT")
            kT = pp.tile([128, B * T], F16, tag="kT")
            v_sb = pp.tile([128, B * T], F16, tag="v_sb")
            cosT = pp.tile([128, S], F16, tag="cosT")
            ssinT = pp.tile([128, S], F16, tag="ssinT")
            mask_sb = pp.tile([128, 128], F16, tag="masks")
            outT_sb = pp.tile([128, B * GPC * S], F16, tag="outT_sb")
            ident = pp.tile([128, 128], F16, tag="ident")
            ones = pp.tile([128, 1], F16, tag="ones")

            nc.vector.memset(ones[:], 1.0)
            make_identity(nc, ident[:])

            xT_r = xT.rearrange("(k p) t -> p k t", p=128)
            xt_tiles = {}

            def xt_alloc(hn):
                xt_t = xtp.tile([128, KCH * HNW], F16, tag="xt",
                                name=f"xt{hn}")
                xt_tiles[hn] = xt_t
                return xt_t[:].rearrange("p (k t) -> p k t", k=KCH)

            def xt_load(hn, split=4):
                t0 = hn * HNW
                dst = xt_alloc(hn)
                ksz = KCH // split
                for i in range(split):
                    nc.sync.dma_start(
                        dst[:, i * ksz:(i + 1) * ksz, :],
                        xT_r[:, i * ksz:(i + 1) * ksz, t0:t0 + HNW])

            # ---- DMA issue (priority order) ----
            # Stage-0 stream, k-major. First 4 k chunks individually (fast
            # first-matmul), the rest in pairs (fewer HWDGE descriptor
            # fetches). Supply ~2 k per 1.3us vs consumption 2 per 2.6us.
            xt0_dst = xt_alloc(0)
            kk = 0
            while kk < KCH:
                step = 1 if kk < 4 else 2
                nc.sync.dma_start(xt0_dst[:, kk:kk + step, :],
                                  xT_r[:, kk:kk + step, 0:HNW])
                c0 = kk * MCH * 128
                if kk == 0:
                    # m=0 chunk alone so the first matmul starts sooner
                    nc.sync.dma_start(wq_sb[:, 0:128], wqkv[:, 0:128])
                    nc.sync.dma_start(wq_sb[:, 128:MCH * 128],
                                      wqkv[:, 128:MCH * 128])
                else:
                    nc.sync.dma_start(wq_sb[:, c0:c0 + step * MCH * 128],
                                      wqkv[:, c0:c0 + step * MCH * 128])
                kk += step
                if kk == 16:
                    nc.sync.dma_start(cosT[:], cosT_d[:])
                    nc.sync.dma_start(ssinT[:], ssinT_d[:])
                    nc.sync.dma_start(mask_sb[:], mask_d[:])
            # KV cache (b0), then xt1, then KV cache (b1): matches the order
            # stages 1-2 consume them.
            vc_r = vc_d.rearrange("(b tc p) d -> p b tc d", b=B, p=128)

            def kv_load(b):
                nc.sync.dma_start(kT[:, b * T:b * T + P],
                                  kcT_d[:, b * P:(b + 1) * P])
                nc.sync.dma_start(
                    v_sb[:, b * T:b * T + P].rearrange(
                        "p (tc d) -> p tc d", tc=8),
                    vc_r[:, b])

            kv_load(0)
            xt_load(1)

            # ---- emitters -------------------------------------------------
            def rope_chunk(src_ap, c0, s0, swdge):
                """RoPE over one 512-wide chunk at col c0, in place: 2
                half-swap DMAs, then rot *= ssin, src *= cos, src += rot
                (all DVE, no scratch beyond rot)."""
                rot = ropep.tile([128, 512], F16, tag="rot", name="rot")
                eng = nc.gpsimd if swdge else nc.sync
                eng.dma_start(rot[0:64, :], src_ap[64:128, c0:c0 + 512])
                eng.dma_start(rot[64:128, :], src_ap[0:64, c0:c0 + 512])
                nc.vector.tensor_mul(rot[:], rot[:], ssinT[:, s0:s0 + 512])
                nc.vector.tensor_mul(src_ap[:, c0:c0 + 512],
                                     src_ap[:, c0:c0 + 512],
                                     cosT[:, s0:s0 + 512])
                nc.vector.tensor_add(src_ap[:, c0:c0 + 512],
                                     src_ap[:, c0:c0 + 512], rot[:])

            def evac_m(hn, b, s0, m, ps, trp):
                """Evacuate one [128,512] qkv psum group; rope q/k; for v,
                transpose into v_sb (yields after each transpose matmul)."""
                ev_dve = (m % 2 == 1)
                swdge = hn != 0
                if m < GPC:
                    dst = qT[:, m * BS + b * S + s0:
                                m * BS + b * S + s0 + HNW]
                    if ev_dve:
                        nc.vector.tensor_copy(dst, ps[:])
                    else:
                        nc.scalar.copy(dst, ps[:])
                    rope_chunk(qT, m * BS + b * S + s0, s0, swdge)
                elif m == GPC:
                    dst = kT[:, b * T + P + s0:b * T + P + s0 + HNW]
                    nc.scalar.copy(dst, ps[:])
                    rope_chunk(kT, b * T + P + s0, s0, swdge)
                else:
                    vt = vtp.tile([128, HNW], F16, tag="vt", name=f"vt{hn}")
                    nc.vector.tensor_copy(vt[:], ps[:])
                    tr = trp.tile([128, HNW], F16, tag="big" if trp is not None and trp.name == "psbig" else "tr", name=f"tr{hn}")
                    for i in range(HNW // 128):
                        nc.tensor.transpose(
                            tr[:, 128 * i:128 * (i + 1)],
                            vt[:, 128 * i:128 * (i + 1)], ident[:])
                        yield
                    vch0 = 16 * b + 8 + s0 // 128
                    nc.vector.tensor_copy(
                        v_sb[:, 128 * vch0:128 * vch0 + HNW], tr[:])

            def qkv_hn(hn):
                """QKV projection for one 512-token block; m-outer, yields
                after each matmul so attention chunks can interleave."""
                b = hn // (HN // B)
                s0 = (hn % (HN // B)) * HNW   # within-batch token offset
                xt_t = xt_tiles[hn]
                for m in range(MCH):
                    ps = psbig.tile([128, 512], F32, tag="big",
                                    name=f"qkv{hn}_{m}")
                    for k in range(KCH):
                        c0 = (k * MCH + m) * 128
                        nc.tensor.matmul(
                            ps[:], wq_sb[:, c0:c0 + 128],
                            xt_t[:, k * HNW:(k + 1) * HNW],
                            start=(k == 0), stop=(k == KCH - 1))
                        yield
                    yield from evac_m(hn, b, s0, m, ps, psbig)

            def qkv_hn0_kmajor(ps0):
                """Stage-0 QKV for block 0: k-outer over 6 concurrent PSUM
                groups while the DMA stream is the limiter (k < KT), then
                finish each group m-sequentially so evacs+ropes stagger."""
                KT = KCH - 4
                xt_t = xt_tiles[0]
                groups = [ps0.tile([128, 512], F32, tag=f"q{m}",
                                   name=f"qkv0_{m}") for m in range(MCH)]
                for k in range(KT):
                    for m in range(MCH):
                        c0 = (k * MCH + m) * 128
                        nc.tensor.matmul(
                            groups[m][:], wq_sb[:, c0:c0 + 128],
                            xt_t[:, k * HNW:(k + 1) * HNW],
                            start=(k == 0), stop=False)
                for m in (GPC, 0, 1, 2, 3, MCH - 1):
                    for k in range(KT, KCH):
                        c0 = (k * MCH + m) * 128
                        nc.tensor.matmul(
                            groups[m][:], wq_sb[:, c0:c0 + 128],
                            xt_t[:, k * HNW:(k + 1) * HNW],
                            start=False, stop=(k == KCH - 1))
                    for _ in evac_m(0, 0, 0, m, groups[m], ps0):
                        pass

            def load_wo(hb):
                wo_t = wop.tile([128, GPC * HBW], F16, tag="wo",
                                name=f"wo{hb}")
                c0 = hb * GPC * HBW
                nc.sync.dma_start(wo_t[:], wo[:, c0:c0 + GPC * HBW])
                return wo_t

            def oproj_tail(sc_list, preloaded):
                """Tail o_proj pass (b=1): the first 4 groups emit their
                g<3 matmuls up front (they only need earlier finalizes) so
                the PE stays busy through the last block's finalize chain;
                alternates the op/big PSUM rings for depth 4."""
                b = 1
                engs = [nc.scalar, nc.vector]
                wo_tiles = dict(preloaded)

                def ensure(hb):
                    if hb < NHB and hb not in wo_tiles:
                        wo_tiles[hb] = load_wo(hb)

                ensure(0)
                ensure(1)
                units = []
                for hbp in range(NHB // 2):
                    for sc in sc_list:
                        for h2 in range(2):
                            units.append((2 * hbp + h2, sc, hbp, h2))
                NPRO = 4
                pro_ps = []
                for i, (hb, sc, hbp, h2) in enumerate(units[:NPRO]):
                    ops = psbig.tile([128, HBW], F32,
                                     tag="op" if i % 2 == 0 else "big",
                                     name=f"opt{sc}_{hb}")
                    pro_ps.append(ops)
                    for g in range(GPC - 1):
                        lcol = b * GPC * S + g * S + 128 * sc
                        nc.tensor.matmul(
                            ops[:], outT_sb[:, lcol:lcol + 128],
                            wo_tiles[hb][:, g * HBW:(g + 1) * HBW],
                            start=(g == 0), stop=False)
                ys = None
                for i, (hb, sc, hbp, h2) in enumerate(units):
                    if i == 2:
                        ensure(2)
                        ensure(3)
                    elif i == 10:
                        ensure(4)
                        ensure(5)
                    elif i == 18:
                        ensure(6)
                        ensure(7)
                    eng = engs[i % 2]
                    if i < NPRO:
                        ops = pro_ps[i]
                        g0 = GPC - 1
                    else:
                        ops = psbig.tile([128, HBW], F32,
                                         tag="op" if i % 2 == 0 else "big",
                                         name=f"opt{sc}_{hb}")
                        g0 = 0
                    for g in range(g0, GPC):
                        lcol = b * GPC * S + g * S + 128 * sc
                        nc.tensor.matmul(
                            ops[:], outT_sb[:, lcol:lcol + 128],
                            wo_tiles[hb][:, g * HBW:(g + 1) * HBW],
                            start=(g == 0 and g0 == 0),
                            stop=(g == GPC - 1))
                    if h2 == 0:
                        ys = ysp.tile([128, 2 * HBW], F16, tag="ys",
                                      name=f"yst{sc}_{hbp}")
                    dst = ys[:, h2 * HBW:(h2 + 1) * HBW]
                    if eng is nc.scalar:
                        eng.copy(dst, ops[:])
                    else:
                        eng.tensor_copy(dst, ops[:])
                    nc.sync.dma_start(
                        y[b * S + 128 * sc:b * S + 128 * (sc + 1),
                          HBW * hb:HBW * (hb + 1)], dst)

            def oproj_pass(b, sc_list, preloaded=None, single_dma=False):
                """One o_proj pass: hb-pair-outer; per group 4 contraction
                matmuls + evac copy into half a [128,1024] ys tile; one y DMA
                per pair. wo tiles roll through `wop` (SWDGE loads) with a
                one-pair prefetch distance."""
                engs = [nc.scalar, nc.vector]
                i = 0
                wo_tiles = dict(preloaded or {})

                def ensure(hb):
                    if hb < NHB and hb not in wo_tiles:
                        wo_tiles[hb] = load_wo(hb)

                ensure(0)
                ensure(1)
                for hbp in range(NHB // 2):
                    for si, sc in enumerate(sc_list):
                        for h2 in range(2):
                            hb = 2 * hbp + h2
                            eng = engs[i % 2]
                            i += 1

                            def unit(b=b, sc=sc, hb=hb, h2=h2, si=si,
                                     hbp=hbp, eng=eng, wo_t=wo_tiles[hb]):
                                if si == 0 and h2 == 0:
                                    ensure(2 * hbp + 2)
                                    ensure(2 * hbp + 3)
                                ops = psbig.tile([128, HBW], F32, tag="op",
                                                 name=f"op{b}_{sc}_{hb}")
                                for g in range(GPC):
                                    lcol = b * GPC * S + g * S + 128 * sc
                                    nc.tensor.matmul(
                                        ops[:], outT_sb[:, lcol:lcol + 128],
                                        wo_t[:, g * HBW:(g + 1) * HBW],
                                        start=(g == 0), stop=(g == GPC - 1))
                                if h2 == 0:
                                    ys = ysp.tile([128, 2 * HBW], F16,
                                                  tag="ys",
                                                  name=f"ys{b}_{sc}_{hbp}")
                                    oproj_pass.ys = ys
                                ys = oproj_pass.ys
                                dst = ys[:, h2 * HBW:(h2 + 1) * HBW]
                                if eng is nc.scalar:
                                    eng.copy(dst, ops[:])
                                else:
                                    eng.tensor_copy(dst, ops[:])
                                if single_dma:
                                    nc.sync.dma_start(
                                        y[b * S + 128 * sc:
                                          b * S + 128 * (sc + 1),
                                          HBW * hb:HBW * (hb + 1)], dst)
                                elif h2 == 1:
                                    nc.sync.dma_start(
                                        y[b * S + 128 * sc:
                                          b * S + 128 * (sc + 1),
                                          1024 * hbp:1024 * (hbp + 1)],
                                        ys[:])

                            yield unit

            # finalize: normalize one attention block's output.
            # Split in two so PE fillers sit between the sums matmul and
            # the broadcast matmul (which waits on the DVE reciprocal).
            def finalize_a(pend):
                f_acc, f_ot, f_ocol = pend
                sums = psp.tile([128, 512], F32, tag="sc", name="sums")
                nc.tensor.matmul(sums[0:1, :], ones[:], f_acc[:],
                                 start=True, stop=True)
                rc = rcp.tile([1, 512], F16, tag="rc", name="rc")
                with nc.allow_low_precision(reason="softmax denom fits fp16"):
                    nc.vector.reciprocal(rc[:], sums[0:1, :])
                return (rc, f_ot, f_ocol)

            def finalize_b(pend2):
                rc, f_ot, f_ocol = pend2
                bc = bcp.tile([128, 512], F16, tag="bc", name="bc")
                nc.gpsimd.partition_broadcast(bc[:], rc[:])
                nc.vector.tensor_mul(outT_sb[:, f_ocol:f_ocol + 512],
                                     f_ot[:], bc[:])

            pending = [None]

            def attn_block(b, g, j, fillers, cadence):
                """One attention s-block (512 queries): scores+exp+pv over
                n_t kv chunks, pipelined; pulls `cadence` (fractional) filler
                units from `fillers` after each chunk's scores matmul.
                Boundary chunks only compute the visible query subrange."""
                scol = g * BS + b * S + j * 512
                n_t = (P // 128) + 4 * (j + 1)      # causal skip
                acc = accp.tile([128, 512], F16, tag="acc",
                                name=f"acc{b}{g}{j}")
                ot_ps = psp.tile([128, 512], F32, tag="ot",
                                 name=f"ot{b}{g}{j}")
                prev = None
                credit = 0.0
                for ti in range(n_t):
                    if ti < 8:
                        kcol = b * T + 128 * ti
                    else:
                        kcol = b * T + P + 128 * (ti - 8)
                    vch = 16 * b + ti
                    r_idx = (ti - 8) - 4 * j
                    s_lo = 128 * r_idx if (ti >= 8 and 0 <= r_idx < 4) else 0
                    sc_ps = psp.tile([128, 512], F32, tag="sc", name="sc")
                    nc.tensor.matmul(sc_ps[:, s_lo:512],
                                     kT[:, kcol:kcol + 128],
                                     qT[:, scol + s_lo:scol + 512],
                                     start=True, stop=True)
                    pt = ptp.tile([128, 512], F16, tag="pt", name="pt")
                    nc.scalar.activation(pt[:, s_lo:512], sc_ps[:, s_lo:512],
                                         mybir.ActivationFunctionType.Exp,
                                         scale=SCALE)
                    if ti >= 8 and 0 <= r_idx < 4:
                        # diagonal 128-col strip: in-chunk causal triangle
                        nc.vector.tensor_mul(
                            pt[:, s_lo:s_lo + 128], pt[:, s_lo:s_lo + 128],
                            mask_sb[:])
                    if ti == 0:
                        nc.vector.tensor_copy(acc[:], pt[:])
                    else:
                        nc.vector.tensor_add(acc[:, s_lo:512],
                                             pt[:, s_lo:512],
                                             acc[:, s_lo:512])
                    # fillers between the scores and the previous pv;
                    # trimmed chunks leave more PE idle, so weight them up
                    credit += cadence + 3.0 * (s_lo / 512.0)
                    while credit >= 1.0:
                        credit -= 1.0
                        if not next_filler(fillers):
                            break
                    if prev is not None:
                        p_pt, p_vch, p_slo, p_first = prev
                        nc.tensor.matmul(
                            ot_ps[:, p_slo:512],
                            v_sb[:, 128 * p_vch:128 * (p_vch + 1)],
                            p_pt[:, p_slo:512], start=p_first, stop=False)
                    prev = (pt, vch, s_lo, ti == 0)
                    if ti == 0 and pending[0] is not None:
                        attn_block.pend2 = finalize_a(pending[0])
                        pending[0] = None
                    elif ti == 4 and attn_block.pend2 is not None:
                        finalize_b(attn_block.pend2)
                        attn_block.pend2 = None
                p_pt, p_vch, p_slo, p_first = prev
                nc.tensor.matmul(ot_ps[:, p_slo:512],
                                 v_sb[:, 128 * p_vch:128 * (p_vch + 1)],
                                 p_pt[:, p_slo:512], start=p_first, stop=True)
                ocol = b * GPC * S + g * S + j * 512
                pending[0] = (acc, ot_ps, ocol)

            attn_block.pend2 = None

            def next_filler(fillers):
                while fillers:
                    try:
                        u = next(fillers[0])
                        if callable(u):
                            u()
                        return True
                    except StopIteration:
                        fillers.pop(0)
                return False

            def drain(fillers):
                while next_filler(fillers):
                    pass

            # ---- schedule -------------------------------------------------
            # stage 0: qkv(hn0) k-major in its own 6-bank+tr PSUM scope
            with tc.tile_pool(name="ps0", bufs=1, space="PSUM") as ps0:
                qkv_hn0_kmajor(ps0)
            with (tc.tile_pool(name="ps", bufs=2, space="PSUM") as psp,
                  tc.tile_pool(name="psbig", bufs=2, space="PSUM") as psbig):
                # stage 1: attn(b0, j=0) + qkv(hn1). Pre-pull ~36 units so
                # the PE has work while hn0's q/k rope chains complete.
                fill = [qkv_hn(1)]
                for _ in range(32):
                    next_filler(fill)
                for g in range(GPC):
                    attn_block(0, g, 0, fill, cadence=3.2)
                    if g == 1:
                        xt_load(2, split=8)
                drain(fill)
                # stage 2: attn(b0, j=1) + qkv(hn2)
                fill = [qkv_hn(2)]
                for g in range(GPC):
                    attn_block(0, g, 1, fill, cadence=2.85)
                    if g == 1:
                        xt_load(3, split=8)
                    elif g == 2:
                        kv_load(1)
                drain(fill)
                # stage 3: attn(b1, j=0) + qkv(hn3) + o_proj pass A (b0)
                gen_qkv3 = qkv_hn(3)
                genA = oproj_pass(0, range(8))
                fill = [gen_qkv3, genA]
                for g in range(GPC):
                    attn_block(1, g, 0, fill, cadence=4.1)
                drain([gen_qkv3])
                # stage 4: attn(b1, j=1) + o_proj passes A remainder + B.
                # Prime pass B so its first wo tile loads ahead of use.
                genB = oproj_pass(1, range(0, 4))
                fill = [genA, genB]
                for g in range(GPC):
                    attn_block(1, g, 1, fill, cadence=1.2)
                # normalize the last block while leftover fillers keep PE
                # busy, then the b1/j1-dependent o_proj tail
                p2 = finalize_a(pending[0])
                pending[0] = None
                drain(fill)
                finalize_b(p2)
                oproj_tail(range(4, 8), {})

    nc.compile()
    return nc


_PROGRAM = None


def _get_program():
    global _PROGRAM
    if _PROGRAM is None:
        _PROGRAM = _build_program()
    return _PROGRAM


def _shard_inputs(hidden_states, w_qkv, w_o, cos, sin, k_cache, v_cache):
    """Build the 8 per-core input maps (numpy, fp16)."""
    hs = np.asarray(hidden_states, np.float32)
    w_qkv = np.asarray(w_qkv, np.float32)
    w_o = np.asarray(w_o, np.float32)
    cos = np.asarray(cos, np.float32)
    sin = np.asarray(sin, np.float32)
    k_cache = np.asarray(k_cache, np.float32)
    v_cache = np.asarray(v_cache, np.float32)

    xT = np.ascontiguousarray(hs.reshape(BS, H).T.astype(np.float16))
    cosT = np.ascontiguousarray(cos.T.astype(np.float16))
    ssinT = sin.T.astype(np.float16).copy()
    ssinT[0:64] *= -1.0
    ssinT = np.ascontiguousarray(ssinT)

    # lower-triangle tile: mask[t, s] = (s >= t)
    tl = np.arange(128)[:, None]
    sl = np.arange(128)[None, :]
    mask = np.ascontiguousarray((sl >= tl).astype(np.float16))

    in_maps = []
    for c in range(NCORES):
        wq_c = w_qkv[:, c * GPC * D:(c + 1) * GPC * D]
        wk_c = w_qkv[:, NQ * D + c * D:NQ * D + (c + 1) * D]
        wv_c = w_qkv[:, (NQ + NKV) * D + c * D:(NQ + NKV) * D + (c + 1) * D]
        wc = np.concatenate([wq_c, wk_c, wv_c], axis=1)      # [H, 768]
        # k-major: [p, k*MCH*128 + m*128 + col]
        wqkv_r = np.ascontiguousarray(
            wc.reshape(KCH, 128, MCH, 128).transpose(1, 0, 2, 3)
            .reshape(128, KCH * MCH * 128).astype(np.float16))
        wo_c = w_o[c * GPC * D:(c + 1) * GPC * D, :]          # [512, H]
        # hb-major: [p, hb*GPC*512 + g*512 + col]
        wo_r = np.ascontiguousarray(
            wo_c.reshape(GPC, 128, NHB, HBW).transpose(1, 2, 0, 3)
            .reshape(128, NHB * GPC * HBW).astype(np.float16))
        kcT = np.ascontiguousarray(
            k_cache[:, :, c, :].reshape(B * P, D).T.astype(np.float16))
        vc = np.ascontiguousarray(
            v_cache[:, :, c, :].reshape(B * P, D).astype(np.float16))
        in_maps.append(dict(xT=xT, wqkv=wqkv_r, wo=wo_r, cosT=cosT,
                            ssinT=ssinT, kcT=kcT, vc=vc, masks=mask))
    return in_maps


def _run(in_maps, trace=False):
    nc = _get_program()
    return run_bass_kernel_spmd(nc, in_maps, list(range(NCORES)), trace=trace)


def kernel(hidden_states, w_qkv, w_o, cos, sin, k_cache, v_cache):
    in_maps = _shard_inputs(hidden_states, w_qkv, w_o, cos, sin,
                            k_cache, v_cache)
    res = _run(in_maps)
    acc = np.zeros((BS, H), np.float64)
    for c in range(NCORES):
        acc += res.results[c]["y"]
    return acc.astype(np.float32).reshape(B, S, H)


# revision 45
# speedup vs baseline: 1.0087x; 1.0009x over previous
"""Llama GQA attention layer (prefill with KV cache) as a Trainium2 Bass/Tile
kernel, tensor-parallel over heads across 8 NeuronCores.

Contract: kernel(**inputs) takes the FULL unsharded inputs (numpy, fp32) and
returns the FULL [B, S, H] output. Sharding: each core gets 4 q-heads and the
matching kv-head (w_qkv column shard, w_o row shard); hidden_states is
replicated (fed pre-transposed); the o_proj row-parallel all-reduce is a host
numpy sum over the 8 partial outputs.

v3 changes over v2:
- QKV projection on 512-token blocks (N=512 moving) -> half the matmul count.
- Stage 0 is k-outer across 6 concurrent PSUM banks, fed by a k-major DMA
  stream, so the first matmul starts ~2us in and is never DMA-starved.
- Causal triangle trimming: boundary key-chunks only compute the visible
  query subrange; the mask shrinks to one [128,128] lower-triangle tile.
- o_proj iterates hb-outer with rolling [128, 4*512] w_o tiles; y is written
  per (128-token, 1024-col) pair chunk.
- DMA-instruction count trimmed (each HWDGE descriptor fetch serializes for
  ~625ns); rope half-swaps and w_o loads ride the Pool engine's SWDGE path.
- Separate PSUM rings for qkv groups vs o_proj groups; softmax-sum tiles
  share the scores ring; fractional filler cadence avoids end-of-stage
  starvation.

Self-contained: hardcodes all shapes; only imports the toolchain from
/opt/trn_rl_repo.
"""

import sys

if "/opt/trn_rl_repo" not in sys.path:
    sys.path.insert(0, "/opt/trn_rl_repo")

import numpy as np

import concourse.bass as bass
import concourse.mybir as mybir
import concourse.tile as tile
from concourse import bacc
from concourse.bass_utils import run_bass_kernel_spmd
from concourse.masks import make_identity

# Problem shapes
B, S, P = 2, 1024, 1024
T = P + S                      # 2048 total kv positions
H, NQ, NKV, D = 4096, 32, 8, 128
G = NQ // NKV                  # 4 q heads per kv head
NCORES = 8
GPC = NQ // NCORES             # 4 q heads per core
SCALE = 1.0 / float(np.sqrt(D))

BS = B * S                     # 2048 tokens (b-major)
QKV_COLS = GPC * D + 2 * D     # 768 per-core qkv output columns
KCH = 32                       # H // 128 contraction chunks
MCH = QKV_COLS // 128          # 6 output chunks (0-3 q, 4 k, 5 v)
HN = 4                         # 512-token blocks in QKV projection
HNW = BS // HN                 # 512
HBW = 512                      # o_proj output-column chunk
NHB = H // HBW                 # 8 hb chunks
F16 = mybir.dt.float16
F32 = mybir.dt.float32


def _build_program():
    nc = bacc.Bacc("TRN2", target_bir_lowering=False, debug=False,
                   num_devices=NCORES)

    xT = nc.dram_tensor("xT", [H, BS], F16, kind="ExternalInput").ap()
    # wqkv k-major: [128, k*MCH*128 + m*128 + c]
    wqkv = nc.dram_tensor("wqkv", [128, KCH * MCH * 128], F16,
                          kind="ExternalInput").ap()
    # wo hb-major: [128, hb*GPC*512 + g*512 + c]
    wo = nc.dram_tensor("wo", [128, NHB * GPC * HBW], F16,
                        kind="ExternalInput").ap()
    cosT_d = nc.dram_tensor("cosT", [128, S], F16, kind="ExternalInput").ap()
    ssinT_d = nc.dram_tensor("ssinT", [128, S], F16, kind="ExternalInput").ap()
    kcT_d = nc.dram_tensor("kcT", [128, B * P], F16, kind="ExternalInput").ap()
    vc_d = nc.dram_tensor("vc", [B * P, D], F16, kind="ExternalInput").ap()
    mask_d = nc.dram_tensor("masks", [128, 128], F16,
                            kind="ExternalInput").ap()
    y = nc.dram_tensor("y", [BS, H], F16, kind="ExternalOutput").ap()

    with tile.TileContext(nc) as tc:
        with (tc.tile_pool(name="persist", bufs=1) as pp,
              tc.tile_pool(name="xt", bufs=2) as xtp,
              tc.tile_pool(name="rope", bufs=2) as ropep,
              tc.tile_pool(name="vt", bufs=1) as vtp,
              tc.tile_pool(name="pt", bufs=3) as ptp,
              tc.tile_pool(name="accs", bufs=2) as accp,
              tc.tile_pool(name="rc", bufs=1) as rcp,
              tc.tile_pool(name="bc", bufs=2) as bcp,
              tc.tile_pool(name="ys", bufs=3) as ysp,
              tc.tile_pool(name="wop", bufs=5) as wop):
            # Persistent SBUF tensors. Layouts (all [128 partitions, free]):
            #  qT: head-dim on partitions, cols g*2048 + b*1024 + s
            #  kT: cols b*2048 + t  (t<1024 cache, t>=1024 new)
            #  v_sb: [t, d] chunks; chunk (b, tc) at col 128*(16b+tc),
            #        tc 0-7 cache, 8-15 new
            #  outT_sb: cols b*4096 + g*1024 + s
            wq_sb = pp.tile([128, KCH * MCH * 128], F16, tag="wq_sb")
            qT = pp.tile([128, GPC * BS], F16, tag="qT")
            kT = pp.tile([128, B * T], F16, tag="kT")
            v_sb = pp.tile([128, B * T], F16, tag="v_sb")
            cosT = pp.tile([128, S], F16, tag="cosT")
            ssinT = pp.tile([128, S], F16, tag="ssinT")
            mask_sb = pp.tile([128, 128], F16, tag="masks")
            outT_sb = pp.tile([128, B * GPC * S], F16, tag="outT_sb")
            ident = pp.tile([128, 128], F16, tag="ident")
            ones = pp.tile([128, 1], F16, tag="ones")
            ones1 = pp.tile([1, 128], F16, tag="ones1")

            nc.vector.memset(ones[:], 1.0)
            nc.vector.memset(ones1[:], 1.0)
            make_identity(nc, ident[:])

            xT_r = xT.rearrange("(k p) t -> p k t", p=128)
            xt_tiles = {}

            def xt_alloc(hn):
                xt_t = xtp.tile([128, KCH * HNW], F16, tag="xt",
                                name=f"xt{hn}")
                xt_tiles[hn] = xt_t
                return xt_t[:].rearrange("p (k t) -> p k t", k=KCH)

            def xt_load(hn, split=4):
                t0 = hn * HNW
                dst = xt_alloc(hn)
                ksz = KCH // split
                for i in range(split):
                    nc.sync.dma_start(
                        dst[:, i * ksz:(i + 1) * ksz, :],
                        xT_r[:, i * ksz:(i + 1) * ksz, t0:t0 + HNW])

            # ---- DMA issue (priority order) ----
            # Stage-0 stream, k-major. First 4 k chunks individually (fast
            # first-matmul), the rest in pairs (fewer HWDGE descriptor
            # fetches). Supply ~2 k per 1.3us vs consumption 2 per 2.6us.
            xt0_dst = xt_alloc(0)
            kk = 0
            while kk < KCH:
                step = 1 if kk < 4 else 2
                nc.sync.dma_start(xt0_dst[:, kk:kk + step, :],
                                  xT_r[:, kk:kk + step, 0:HNW])
                c0 = kk * MCH * 128
                if kk == 0:
                    # m=0 chunk alone so the first matmul starts sooner
                    nc.sync.dma_start(wq_sb[:, 0:128], wqkv[:, 0:128])
                    nc.sync.dma_start(wq_sb[:, 128:MCH * 128],
                                      wqkv[:, 128:MCH * 128])
                else:
                    nc.sync.dma_start(wq_sb[:, c0:c0 + step * MCH * 128],
                                      wqkv[:, c0:c0 + step * MCH * 128])
                kk += step
                if kk == 16:
                    nc.sync.dma_start(cosT[:], cosT_d[:])
                    nc.sync.dma_start(ssinT[:], ssinT_d[:])
                    nc.sync.dma_start(mask_sb[:], mask_d[:])
            # KV cache (b0), then xt1, then KV cache (b1): matches the order
            # stages 1-2 consume them.
            vc_r = vc_d.rearrange("(b tc p) d -> p b tc d", b=B, p=128)

            def kv_load(b):
                nc.sync.dma_start(kT[:, b * T:b * T + P],
                                  kcT_d[:, b * P:(b + 1) * P])
                nc.sync.dma_start(
                    v_sb[:, b * T:b * T + P].rearrange(
                        "p (tc d) -> p tc d", tc=8),
                    vc_r[:, b])

            kv_load(0)
            xt_load(1)

            # ---- emitters -------------------------------------------------
            def rope_swap(src_ap, c0, swdge):
                """Issue the half-swap DMAs for one 512-wide chunk; the
                muls are emitted later (rope_muls) so their ~2.4us DMA
                latency never stalls the in-order DVE queue."""
                rot = ropep.tile([128, 512], F16, tag="rot", name="rot")
                eng = nc.gpsimd if swdge else nc.sync
                eng.dma_start(rot[0:64, :], src_ap[64:128, c0:c0 + 512])
                eng.dma_start(rot[64:128, :], src_ap[0:64, c0:c0 + 512])
                return rot

            def rope_muls(src_ap, rot, c0, s0):
                nc.vector.tensor_mul(rot[:], rot[:], ssinT[:, s0:s0 + 512])
                nc.vector.tensor_mul(src_ap[:, c0:c0 + 512],
                                     src_ap[:, c0:c0 + 512],
                                     cosT[:, s0:s0 + 512])
                nc.vector.tensor_add(src_ap[:, c0:c0 + 512],
                                     src_ap[:, c0:c0 + 512], rot[:])

            def rope_chunk(src_ap, c0, s0, swdge, ps=None):
                rope_muls(src_ap, rope_swap(src_ap, c0, swdge), c0, s0)

            def evac_m(hn, b, s0, m, ps, trp):
                """Evacuate one [128,512] qkv psum group; rope q/k; for v,
                transpose into v_sb (yields after each transpose matmul)."""
                swdge = hn != 0
                if m < GPC:
                    dst = qT[:, m * BS + b * S + s0:
                                m * BS + b * S + s0 + HNW]
                    if m % 2 == 0:
                        nc.scalar.copy(dst, ps[:])
                    else:
                        nc.vector.tensor_copy(dst, ps[:])
                    rope_chunk(qT, m * BS + b * S + s0, s0, swdge)
                elif m == GPC:
                    dst = kT[:, b * T + P + s0:b * T + P + s0 + HNW]
                    nc.scalar.copy(dst, ps[:])
                    rope_chunk(kT, b * T + P + s0, s0, swdge)
                else:
                    vt = vtp.tile([128, HNW], F16, tag="vt", name=f"vt{hn}")
                    nc.vector.tensor_copy(vt[:], ps[:])
                    tr = trp.tile([128, HNW], F16,
                                  tag="tr" if trp.name == "ps0" else "big",
                                  name=f"tr{hn}")
                    for i in range(HNW // 128):
                        nc.tensor.transpose(
                            tr[:, 128 * i:128 * (i + 1)],
                            vt[:, 128 * i:128 * (i + 1)], ident[:])
                        yield
                    vch0 = 16 * b + 8 + s0 // 128
                    nc.vector.tensor_copy(
                        v_sb[:, 128 * vch0:128 * vch0 + HNW], tr[:])

            def qkv_hn(hn):
                """QKV projection for one 512-token block; m-outer, yields
                after each matmul so attention chunks can interleave."""
                b = hn // (HN // B)
                s0 = (hn % (HN // B)) * HNW   # within-batch token offset
                xt_t = xt_tiles[hn]
                for m in range(MCH):
                    ps = bigp[0].tile([128, 512], F32, tag="big",
                                      name=f"qkv{hn}_{m}")
                    for k in range(KCH):
                        c0 = (k * MCH + m) * 128
                        nc.tensor.matmul(
                            ps[:], wq_sb[:, c0:c0 + 128],
                            xt_t[:, k * HNW:(k + 1) * HNW],
                            start=(k == 0), stop=(k == KCH - 1))
                        yield
                    yield from evac_m(hn, b, s0, m, ps, bigp[0])

            def qkv_hn0_kmajor(ps0):
                """Stage-0 QKV for block 0: k-outer over 6 concurrent PSUM
                groups while the DMA stream is the limiter (k < KT), then
                finish each group m-sequentially so evacs+ropes stagger."""
                KT = KCH - 8
                xt_t = xt_tiles[0]
                groups = [ps0.tile([128, 512], F32, tag=f"q{m}",
                                   name=f"qkv0_{m}") for m in range(MCH)]
                for k in range(KT):
                    for m in range(MCH):
                        c0 = (k * MCH + m) * 128
                        nc.tensor.matmul(
                            groups[m][:], wq_sb[:, c0:c0 + 128],
                            xt_t[:, k * HNW:(k + 1) * HNW],
                            start=(k == 0), stop=False)
                for m in (GPC, 0, 1, 2, 3, MCH - 1):  # k, q0-3, v
                    for k in range(KT, KCH):
                        c0 = (k * MCH + m) * 128
                        nc.tensor.matmul(
                            groups[m][:], wq_sb[:, c0:c0 + 128],
                            xt_t[:, k * HNW:(k + 1) * HNW],
                            start=False, stop=(k == KCH - 1))
                    for _ in evac_m(0, 0, 0, m, groups[m], ps0):
                        pass

            def load_wo(hb):
                wo_t = wop.tile([128, GPC * HBW], F16, tag="wo",
                                name=f"wo{hb}")
                c0 = hb * GPC * HBW
                nc.sync.dma_start(wo_t[:], wo[:, c0:c0 + GPC * HBW])
                return wo_t

            def oproj_tail(sc_list, preloaded):
                """Tail o_proj pass (b=1): the first 4 groups emit their
                g<3 matmuls up front (they only need earlier finalizes) so
                the PE stays busy through the last block's finalize chain;
                alternates the op/big PSUM rings for depth 4."""
                b = 1
                engs = [nc.scalar, nc.vector]
                wo_tiles = dict(preloaded)

                def ensure(hb):
                    if hb < NHB and hb not in wo_tiles:
                        wo_tiles[hb] = load_wo(hb)

                ensure(0)
                ensure(1)
                units = []
                for hbp in range(NHB // 2):
                    for sc in sc_list:
                        for h2 in range(2):
                            units.append((2 * hbp + h2, sc, hbp, h2))
                NPRO = 4
                pro_ps = []
                for i, (hb, sc, hbp, h2) in enumerate(units[:NPRO]):
                    pool = opp[0] if i % 2 == 0 else bigp[0]
                    ops = pool.tile([128, HBW], F32,
                                    tag="op" if i % 2 == 0 else "big",
                                    name=f"opt{sc}_{hb}")
                    pro_ps.append(ops)
                    for g in range(GPC - 1):
                        lcol = b * GPC * S + g * S + 128 * sc
                        nc.tensor.matmul(
                            ops[:], outT_sb[:, lcol:lcol + 128],
                            wo_tiles[hb][:, g * HBW:(g + 1) * HBW],
                            start=(g == 0), stop=False)
                ys = None
                for i, (hb, sc, hbp, h2) in enumerate(units):
                    if i == 2:
                        ensure(2)
                        ensure(3)
                    elif i == 10:
                        ensure(4)
                        ensure(5)
                    elif i == 18:
                        ensure(6)
                        ensure(7)
                    eng = engs[i % 2]
                    if i < NPRO:
                        ops = pro_ps[i]
                        g0 = GPC - 1
                    else:
                        pool = opp[0] if i % 2 == 0 else bigp[0]
                        ops = pool.tile([128, HBW], F32,
                                        tag="op" if i % 2 == 0 else "big",
                                        name=f"opt{sc}_{hb}")
                        g0 = 0
                    for g in range(g0, GPC):
                        lcol = b * GPC * S + g * S + 128 * sc
                        nc.tensor.matmul(
                            ops[:], outT_sb[:, lcol:lcol + 128],
                            wo_tiles[hb][:, g * HBW:(g + 1) * HBW],
                            start=(g == 0 and g0 == 0),
                            stop=(g == GPC - 1))
                    if h2 == 0:
                        ys = ysp.tile([128, 2 * HBW], F16, tag="ys",
                                      name=f"yst{sc}_{hbp}")
                    dst = ys[:, h2 * HBW:(h2 + 1) * HBW]
                    if eng is nc.scalar:
                        eng.copy(dst, ops[:])
                    else:
                        eng.tensor_copy(dst, ops[:])
                    nc.sync.dma_start(
                        y[b * S + 128 * sc:b * S + 128 * (sc + 1),
                          HBW * hb:HBW * (hb + 1)], dst)

            def oproj_pass(b, sc_list, preloaded=None, single_dma=False,
                           tail_pre=None):
                """One o_proj pass: hb-pair-outer; per group 4 contraction
                matmuls + evac copy into half a [128,1024] ys tile; one y DMA
                per pair. wo tiles roll through `wop` (SWDGE loads) with a
                one-pair prefetch distance."""
                engs = [nc.scalar, nc.vector]
                i = 0
                wo_tiles = dict(preloaded or {})

                def ensure(hb):
                    if hb < NHB and hb not in wo_tiles:
                        wo_tiles[hb] = load_wo(hb)

                ensure(0)
                ensure(1)
                for hbp in range(NHB // 2):
                    for si, sc in enumerate(sc_list):
                        for h2 in range(2):
                            hb = 2 * hbp + h2
                            eng = engs[i % 2]
                            i += 1

                            def unit(b=b, sc=sc, hb=hb, h2=h2, si=si,
                                     hbp=hbp, eng=eng, wo_t=wo_tiles[hb]):
                                if si == 0 and h2 == 0:
                                    ensure(2 * hbp + 2)
                                    ensure(2 * hbp + 3)
                                    if tail_pre is not None \
                                            and hbp == NHB // 2 - 1:
                                        tail_pre[0] = load_wo(0)
                                        tail_pre[1] = load_wo(1)
                                ops = opp[0].tile([128, HBW], F32, tag="op",
                                                  name=f"op{b}_{sc}_{hb}")
                                for g in range(GPC):
                                    lcol = b * GPC * S + g * S + 128 * sc
                                    nc.tensor.matmul(
                                        ops[:], outT_sb[:, lcol:lcol + 128],
                                        wo_t[:, g * HBW:(g + 1) * HBW],
                                        start=(g == 0), stop=(g == GPC - 1))
                                if h2 == 0:
                                    ys = ysp.tile([128, 2 * HBW], F16,
                                                  tag="ys",
                                                  name=f"ys{b}_{sc}_{hbp}")
                                    oproj_pass.ys = ys
                                ys = oproj_pass.ys
                                dst = ys[:, h2 * HBW:(h2 + 1) * HBW]
                                if eng is nc.scalar:
                                    eng.copy(dst, ops[:])
                                else:
                                    eng.tensor_copy(dst, ops[:])
                                if single_dma:
                                    nc.sync.dma_start(
                                        y[b * S + 128 * sc:
                                          b * S + 128 * (sc + 1),
                                          HBW * hb:HBW * (hb + 1)], dst)
                                elif h2 == 1:
                                    nc.sync.dma_start(
                                        y[b * S + 128 * sc:
                                          b * S + 128 * (sc + 1),
                                          1024 * hbp:1024 * (hbp + 1)],
                                        ys[:])

                            yield unit

            # finalize: normalize one attention block's output.
            # Split in two so PE fillers sit between the sums matmul and
            # the broadcast matmul (which waits on the DVE reciprocal).
            def finalize_a(pend):
                f_acc, f_ot, f_ocol = pend
                sums = psp.tile([128, 512], F32, tag="sc", name="sums")
                nc.tensor.matmul(sums[0:1, :], ones[:], f_acc[:],
                                 start=True, stop=True)
                rc = rcp.tile([1, 512], F16, tag="rc", name="rc")
                with nc.allow_low_precision(reason="softmax denom fits fp16"):
                    nc.vector.reciprocal(rc[:], sums[0:1, :])
                return (rc, f_ot, f_ocol)

            def finalize_b(pend2):
                rc, f_ot, f_ocol = pend2
                bc = bcp.tile([128, 512], F16, tag="bc", name="bc")
                nc.gpsimd.partition_broadcast(bc[:], rc[:])
                nc.vector.tensor_mul(outT_sb[:, f_ocol:f_ocol + 512],
                                     f_ot[:], bc[:])

            pending = [None]

            def attn_block(b, g, j, fillers, cadence):
                """One attention s-block (512 queries): scores+exp+pv over
                n_t kv chunks, pipelined; pulls `cadence` (fractional) filler
                units from `fillers` after each chunk's scores matmul.
                Boundary chunks only compute the visible query subrange."""
                scol = g * BS + b * S + j * 512
                n_t = (P // 128) + 4 * (j + 1)      # causal skip
                acc = accp.tile([128, 512], F16, tag="acc",
                                name=f"acc{b}{g}{j}")
                ot_ps = psp.tile([128, 512], F32, tag="ot",
                                 name=f"ot{b}{g}{j}")
                prev = None
                credit = 0.0
                for ti in range(n_t):
                    if ti < 8:
                        kcol = b * T + 128 * ti
                    else:
                        kcol = b * T + P + 128 * (ti - 8)
                    vch = 16 * b + ti
                    r_idx = (ti - 8) - 4 * j
                    s_lo = 128 * r_idx if (ti >= 8 and 0 <= r_idx < 4) else 0
                    sc_ps = psp.tile([128, 512], F32, tag="sc", name="sc")
                    nc.tensor.matmul(sc_ps[:, s_lo:512],
                                     kT[:, kcol:kcol + 128],
                                     qT[:, scol + s_lo:scol + 512],
                                     start=True, stop=True)
                    pt = ptp.tile([128, 512], F16, tag="pt", name="pt")
                    nc.scalar.activation(pt[:, s_lo:512], sc_ps[:, s_lo:512],
                                         mybir.ActivationFunctionType.Exp,
                                         scale=SCALE)
                    if ti >= 8 and 0 <= r_idx < 4:
                        # diagonal 128-col strip: in-chunk causal triangle
                        nc.vector.tensor_mul(
                            pt[:, s_lo:s_lo + 128], pt[:, s_lo:s_lo + 128],
                            mask_sb[:])
                    if ti == 0:
                        nc.vector.tensor_copy(acc[:], pt[:])
                    else:
                        nc.vector.tensor_add(acc[:, s_lo:512],
                                             pt[:, s_lo:512],
                                             acc[:, s_lo:512])
                    # fillers between the scores and the previous pv;
                    # trimmed chunks leave more PE idle, so weight them up
                    credit += cadence + 3.0 * (s_lo / 512.0)
                    while credit >= 1.0:
                        credit -= 1.0
                        if not next_filler(fillers):
                            break
                    if prev is not None:
                        p_pt, p_vch, p_slo, p_first = prev
                        nc.tensor.matmul(
                            ot_ps[:, p_slo:512],
                            v_sb[:, 128 * p_vch:128 * (p_vch + 1)],
                            p_pt[:, p_slo:512], start=p_first, stop=False)
                    prev = (pt, vch, s_lo, ti == 0)
                    if ti == 0 and pending[0] is not None:
                        attn_block.pend2 = finalize_a(pending[0])
                        pending[0] = None
                    elif ti == 4 and attn_block.pend2 is not None:
                        finalize_b(attn_block.pend2)
                        attn_block.pend2 = None
                p_pt, p_vch, p_slo, p_first = prev
                nc.tensor.matmul(ot_ps[:, p_slo:512],
                                 v_sb[:, 128 * p_vch:128 * (p_vch + 1)],
                                 p_pt[:, p_slo:512], start=p_first, stop=True)
                ocol = b * GPC * S + g * S + j * 512
                pending[0] = (acc, ot_ps, ocol)

            attn_block.pend2 = None

            def next_filler(fillers):
                while fillers:
                    try:
                        u = next(fillers[0])
                        if callable(u):
                            u()
                        return True
                    except StopIteration:
                        fillers.pop(0)
                return False

            def drain(fillers):
                while next_filler(fillers):
                    pass

            # ---- schedule -------------------------------------------------
            # stage 0: qkv(hn0) k-major in its own 6-bank+tr PSUM scope
            with tc.tile_pool(name="ps0", bufs=1, space="PSUM") as ps0:
                qkv_hn0_kmajor(ps0)
            # Gate the xt2 DMA pieces on hn0's last rope: its half-swap DMAs
            # share the DMA engines with xt2's 4MB, and losing that race
            # stalls stage-1 attention. Tiny copies (rope-dependent) into the
            # xt slot xt2 will reuse push xt2 behind the swaps.
            m3c = 3 * BS
            xt0_t = xt_tiles[0]
            for i in range(8):
                nc.vector.tensor_copy(
                    xt0_t[0:1, i * (KCH * HNW // 8):i * (KCH * HNW // 8) + 1],
                    qT[0:1, m3c:m3c + 1])
            bigp = [None]
            opp = [None]
            with tc.tile_pool(name="ps", bufs=2, space="PSUM") as psp:
                with tc.tile_pool(name="psbig12", bufs=2,
                                  space="PSUM") as big12:
                    bigp[0] = big12
                    # stage 1: attn(b0, j=0) + qkv(hn1). Pre-pull ~32 units
                    # so the PE has work while hn0's rope chains complete.
                    fill = [qkv_hn(1)]
                    for _ in range(32):
                        next_filler(fill)
                    for g in range(GPC):
                        attn_block(0, g, 0, fill, cadence=3.2)
                        if g == 2:
                            xt_load(2, split=8)
                    drain(fill)
                    # stage 2: attn(b0, j=1) + qkv(hn2)
                    fill = [qkv_hn(2)]
                    for g in range(GPC):
                        attn_block(0, g, 1, fill, cadence=2.85)
                        if g == 1:
                            xt_load(3, split=8)
                        elif g == 2:
                            kv_load(1)
                    drain(fill)
                # stage 3: attn(b1, j=0) + qkv(hn3) + o_proj pass A (b0)
                big34 = tc.alloc_tile_pool(name="psbig", bufs=2,
                                           space="PSUM")
                op34 = tc.alloc_tile_pool(name="psop", bufs=2, space="PSUM")
                bigp[0] = big34
                opp[0] = op34
                gen_qkv3 = qkv_hn(3)
                genA = oproj_pass(0, range(8))
                fill = [gen_qkv3, genA]
                for g in range(GPC):
                    attn_block(1, g, 0, fill, cadence=4.1)
                drain([gen_qkv3])
                # stage 4: attn(b1, j=1) + o_proj passes A remainder + B.
                # Prime pass B so its first wo tile loads ahead of use.
                tail_pre = {}
                genB = oproj_pass(1, range(0, 4), tail_pre=tail_pre)
                fill = [genA, genB]
                for g in range(GPC):
                    attn_block(1, g, 1, fill, cadence=1.2)
                # normalize the last block while leftover fillers keep PE
                # busy, then the b1/j1-dependent o_proj tail
                p2 = finalize_a(pending[0])
                pending[0] = None
                drain(fill)
                finalize_b(p2)
                oproj_tail(range(4, 8), tail_pre)
                op34.release()
                big34.release()

    nc.compile()
    return nc


_PROGRAM = None


def _get_program():
    global _PROGRAM
    if _PROGRAM is None:
        _PROGRAM = _build_program()
    return _PROGRAM


def _shard_inputs(hidden_states, w_qkv, w_o, cos, sin, k_cache, v_cache):
    """Build the 8 per-core input maps (numpy, fp16)."""
    hs = np.asarray(hidden_states, np.float32)
    w_qkv = np.asarray(w_qkv, np.float32)
    w_o = np.asarray(w_o, np.float32)
    cos = np.asarray(cos, np.float32)
    sin = np.asarray(sin, np.float32)
    k_cache = np.asarray(k_cache, np.float32)
    v_cache = np.asarray(v_cache, np.float32)

    xT = np.ascontiguousarray(hs.reshape(BS, H).T.astype(np.float16))
    cosT = np.ascontiguousarray(cos.T.astype(np.float16))
    ssinT = sin.T.astype(np.float16).copy()
    ssinT[0:64] *= -1.0
    ssinT = np.ascontiguousarray(ssinT)

    # lower-triangle tile: mask[t, s] = (s >= t)
    tl = np.arange(128)[:, None]
    sl = np.arange(128)[None, :]
    mask = np.ascontiguousarray((sl >= tl).astype(np.float16))

    in_maps = []
    for c in range(NCORES):
        wq_c = w_qkv[:, c * GPC * D:(c + 1) * GPC * D]
        wk_c = w_qkv[:, NQ * D + c * D:NQ * D + (c + 1) * D]
        wv_c = w_qkv[:, (NQ + NKV) * D + c * D:(NQ + NKV) * D + (c + 1) * D]
        wc = np.concatenate([wq_c, wk_c, wv_c], axis=1)      # [H, 768]
        # k-major: [p, k*MCH*128 + m*128 + col]
        wqkv_r = np.ascontiguousarray(
            wc.reshape(KCH, 128, MCH, 128).transpose(1, 0, 2, 3)
            .reshape(128, KCH * MCH * 128).astype(np.float16))
        wo_c = w_o[c * GPC * D:(c + 1) * GPC * D, :]          # [512, H]
        # hb-major: [p, hb*GPC*512 + g*512 + col]
        wo_r = np.ascontiguousarray(
            wo_c.reshape(GPC, 128, NHB, HBW).transpose(1, 2, 0, 3)
            .reshape(128, NHB * GPC * HBW).astype(np.float16))
        kcT = np.ascontiguousarray(
            k_cache[:, :, c, :].reshape(B * P, D).T.astype(np.float16))
        vc = np.ascontiguousarray(
            v_cache[:, :, c, :].reshape(B * P, D).astype(np.float16))
        in_maps.append(dict(xT=xT, wqkv=wqkv_r, wo=wo_r, cosT=cosT,
                            ssinT=ssinT, kcT=kcT, vc=vc, masks=mask))
    return in_maps


def _run(in_maps, trace=False):
    nc = _get_program()
    return run_bass_kernel_spmd(nc, in_maps, list(range(NCORES)), trace=trace)


def kernel(hidden_states, w_qkv, w_o, cos, sin, k_cache, v_cache):
    in_maps = _shard_inputs(hidden_states, w_qkv, w_o, cos, sin,
                            k_cache, v_cache)
    res = _run(in_maps)
    acc = np.zeros((BS, H), np.float64)
    for c in range(NCORES):
        acc += res.results[c]["y"]
    return acc.astype(np.float32).reshape(B, S, H)


# revision 52
# speedup vs baseline: 1.0167x; 1.0079x over previous
"""Llama GQA attention layer (prefill with KV cache) as a Trainium2 Bass/Tile
kernel, tensor-parallel over heads across 8 NeuronCores.

Contract: kernel(**inputs) takes the FULL unsharded inputs (numpy, fp32) and
returns the FULL [B, S, H] output. Sharding: each core gets 4 q-heads and the
matching kv-head (w_qkv column shard, w_o row shard); hidden_states is
replicated (fed pre-transposed); the o_proj row-parallel all-reduce is a host
numpy sum over the 8 partial outputs.

v3 changes over v2:
- QKV projection on 512-token blocks (N=512 moving) -> half the matmul count.
- Stage 0 is k-outer across 6 concurrent PSUM banks, fed by a k-major DMA
  stream, so the first matmul starts ~2us in and is never DMA-starved.
- Causal triangle trimming: boundary key-chunks only compute the visible
  query subrange; the mask shrinks to one [128,128] lower-triangle tile.
- o_proj iterates hb-outer with rolling [128, 4*512] w_o tiles; y is written
  per (128-token, 1024-col) pair chunk.
- DMA-instruction count trimmed (each HWDGE descriptor fetch serializes for
  ~625ns); rope half-swaps and w_o loads ride the Pool engine's SWDGE path.
- Separate PSUM rings for qkv groups vs o_proj groups; softmax-sum tiles
  share the scores ring; fractional filler cadence avoids end-of-stage
  starvation.

Self-contained: hardcodes all shapes; only imports the toolchain from
/opt/trn_rl_repo.
"""

import sys

if "/opt/trn_rl_repo" not in sys.path:
    sys.path.insert(0, "/opt/trn_rl_repo")

import numpy as np

import concourse.bass as bass
import concourse.mybir as mybir
import concourse.tile as tile
from concourse import bacc
from concourse.bass_utils import run_bass_kernel_spmd
from concourse.masks import make_identity

# Problem shapes
B, S, P = 2, 1024, 1024
T = P + S                      # 2048 total kv positions
H, NQ, NKV, D = 4096, 32, 8, 128
G = NQ // NKV                  # 4 q heads per kv head
NCORES = 8
GPC = NQ // NCORES             # 4 q heads per core
SCALE = 1.0 / float(np.sqrt(D))

BS = B * S                     # 2048 tokens (b-major)
QKV_COLS = GPC * D + 2 * D     # 768 per-core qkv output columns
KCH = 32                       # H // 128 contraction chunks
MCH = QKV_COLS // 128          # 6 output chunks (0-3 q, 4 k, 5 v)
HN = 4                         # 512-token blocks in QKV projection
HNW = BS // HN                 # 512
HBW = 512                      # o_proj output-column chunk
NHB = H // HBW                 # 8 hb chunks
F16 = mybir.dt.float16
F32 = mybir.dt.float32


def _build_program():
    nc = bacc.Bacc("TRN2", target_bir_lowering=False, debug=False,
                   num_devices=NCORES)

    xT = nc.dram_tensor("xT", [H, BS], F16, kind="ExternalInput").ap()
    # wqkv k-major: [128, k*MCH*128 + m*128 + c]
    wqkv = nc.dram_tensor("wqkv", [128, KCH * MCH * 128], F16,
                          kind="ExternalInput").ap()
    # wo hb-major: [128, hb*GPC*512 + g*512 + c]
    wo = nc.dram_tensor("wo", [128, NHB * GPC * HBW], F16,
                        kind="ExternalInput").ap()
    cosT_d = nc.dram_tensor("cosT", [128, S], F16, kind="ExternalInput").ap()
    ssinT_d = nc.dram_tensor("ssinT", [128, S], F16, kind="ExternalInput").ap()
    kcT_d = nc.dram_tensor("kcT", [128, B * P], F16, kind="ExternalInput").ap()
    vc_d = nc.dram_tensor("vc", [B * P, D], F16, kind="ExternalInput").ap()
    mask_d = nc.dram_tensor("masks", [128, 128], F16,
                            kind="ExternalInput").ap()
    y = nc.dram_tensor("y", [BS, H], F16, kind="ExternalOutput").ap()

    with tile.TileContext(nc) as tc:
        with (tc.tile_pool(name="persist", bufs=1) as pp,
              tc.tile_pool(name="xt", bufs=2) as xtp,
              tc.tile_pool(name="rope", bufs=2) as ropep,
              tc.tile_pool(name="vt", bufs=1) as vtp,
              tc.tile_pool(name="pt", bufs=3) as ptp,
              tc.tile_pool(name="accs", bufs=2) as accp,
              tc.tile_pool(name="rc", bufs=1) as rcp,
              tc.tile_pool(name="bc", bufs=2) as bcp,
              tc.tile_pool(name="ys", bufs=3) as ysp,
              tc.tile_pool(name="wop", bufs=5) as wop):
            # Persistent SBUF tensors. Layouts (all [128 partitions, free]):
            #  qT: head-dim on partitions, cols g*2048 + b*1024 + s
            #  kT: cols b*2048 + t  (t<1024 cache, t>=1024 new)
            #  v_sb: [t, d] chunks; chunk (b, tc) at col 128*(16b+tc),
            #        tc 0-7 cache, 8-15 new
            #  outT_sb: cols b*4096 + g*1024 + s
            wq_sb = pp.tile([128, KCH * MCH * 128], F16, tag="wq_sb")
            qT = pp.tile([128, GPC * BS], F16, tag="qT")
            kT = pp.tile([128, B * T], F16, tag="kT")
            v_sb = pp.tile([128, B * T], F16, tag="v_sb")
            cosT = pp.tile([128, S], F16, tag="cosT")
            ssinT = pp.tile([128, S], F16, tag="ssinT")
            mask_sb = pp.tile([128, 128], F16, tag="masks")
            outT_sb = pp.tile([128, B * GPC * S], F16, tag="outT_sb")
            ident = pp.tile([128, 128], F16, tag="ident")
            ones = pp.tile([128, 1], F16, tag="ones")
            ones1 = pp.tile([1, 128], F16, tag="ones1")

            nc.vector.memset(ones[:], 1.0)
            nc.vector.memset(ones1[:], 1.0)
            make_identity(nc, ident[:])

            xT_r = xT.rearrange("(k p) t -> p k t", p=128)
            xt_tiles = {}

            def xt_alloc(hn):
                xt_t = xtp.tile([128, KCH * HNW], F16, tag="xt",
                                name=f"xt{hn}")
                xt_tiles[hn] = xt_t
                return xt_t[:].rearrange("p (k t) -> p k t", k=KCH)

            def xt_load(hn, split=4):
                t0 = hn * HNW
                dst = xt_alloc(hn)
                ksz = KCH // split
                for i in range(split):
                    nc.sync.dma_start(
                        dst[:, i * ksz:(i + 1) * ksz, :],
                        xT_r[:, i * ksz:(i + 1) * ksz, t0:t0 + HNW])

            # ---- DMA issue (priority order) ----
            # Stage-0 stream, k-major. First 4 k chunks individually (fast
            # first-matmul), the rest in pairs (fewer HWDGE descriptor
            # fetches). Supply ~2 k per 1.3us vs consumption 2 per 2.6us.
            xt0_dst = xt_alloc(0)
            kk = 0
            while kk < KCH:
                step = 1 if kk < 4 else 2
                nc.sync.dma_start(xt0_dst[:, kk:kk + step, :],
                                  xT_r[:, kk:kk + step, 0:HNW])
                c0 = kk * MCH * 128
                if kk == 0:
                    # m=0 chunk alone so the first matmul starts sooner
                    nc.sync.dma_start(wq_sb[:, 0:128], wqkv[:, 0:128])
                    nc.sync.dma_start(wq_sb[:, 128:MCH * 128],
                                      wqkv[:, 128:MCH * 128])
                else:
                    nc.sync.dma_start(wq_sb[:, c0:c0 + step * MCH * 128],
                                      wqkv[:, c0:c0 + step * MCH * 128])
                kk += step
                if kk == 16:
                    nc.sync.dma_start(cosT[:], cosT_d[:])
                    nc.sync.dma_start(ssinT[:], ssinT_d[:])
                    nc.sync.dma_start(mask_sb[:], mask_d[:])
            # KV cache (b0), then xt1, then KV cache (b1): matches the order
            # stages 1-2 consume them.
            vc_r = vc_d.rearrange("(b tc p) d -> p b tc d", b=B, p=128)

            def kv_load(b):
                nc.sync.dma_start(kT[:, b * T:b * T + P],
                                  kcT_d[:, b * P:(b + 1) * P])
                nc.sync.dma_start(
                    v_sb[:, b * T:b * T + P].rearrange(
                        "p (tc d) -> p tc d", tc=8),
                    vc_r[:, b])

            kv_load(0)
            xt_load(1)

            # ---- emitters -------------------------------------------------
            def rope_swap(src_ap, c0, swdge):
                """Issue the half-swap DMAs for one 512-wide chunk; the
                muls are emitted later (rope_muls) so their ~2.4us DMA
                latency never stalls the in-order DVE queue."""
                rot = ropep.tile([128, 512], F16, tag="rot", name="rot")
                eng = nc.gpsimd if swdge else nc.sync
                eng.dma_start(rot[0:64, :], src_ap[64:128, c0:c0 + 512])
                eng.dma_start(rot[64:128, :], src_ap[0:64, c0:c0 + 512])
                return rot

            def rope_muls(src_ap, rot, c0, s0):
                nc.vector.tensor_mul(rot[:], rot[:], ssinT[:, s0:s0 + 512])
                nc.vector.tensor_mul(src_ap[:, c0:c0 + 512],
                                     src_ap[:, c0:c0 + 512],
                                     cosT[:, s0:s0 + 512])
                nc.vector.tensor_add(src_ap[:, c0:c0 + 512],
                                     src_ap[:, c0:c0 + 512], rot[:])

            def rope_chunk(src_ap, c0, s0, swdge, ps=None):
                rope_muls(src_ap, rope_swap(src_ap, c0, swdge), c0, s0)

            def evac_m(hn, b, s0, m, ps, trp):
                """Evacuate one [128,512] qkv psum group; rope q/k; for v,
                transpose into v_sb (yields after each transpose matmul)."""
                swdge = hn != 0
                if m < GPC:
                    dst = qT[:, m * BS + b * S + s0:
                                m * BS + b * S + s0 + HNW]
                    if m % 2 == 0:
                        nc.scalar.copy(dst, ps[:])
                    else:
                        nc.vector.tensor_copy(dst, ps[:])
                    rope_chunk(qT, m * BS + b * S + s0, s0, swdge)
                elif m == GPC:
                    dst = kT[:, b * T + P + s0:b * T + P + s0 + HNW]
                    nc.scalar.copy(dst, ps[:])
                    rope_chunk(kT, b * T + P + s0, s0, swdge)
                else:
                    vt = vtp.tile([128, HNW], F16, tag="vt", name=f"vt{hn}")
                    nc.vector.tensor_copy(vt[:], ps[:])
                    tr = trp.tile([128, HNW], F16,
                                  tag="tr" if trp.name == "ps0" else "big",
                                  name=f"tr{hn}")
                    for i in range(HNW // 128):
                        nc.tensor.transpose(
                            tr[:, 128 * i:128 * (i + 1)],
                            vt[:, 128 * i:128 * (i + 1)], ident[:])
                        yield
                    vch0 = 16 * b + 8 + s0 // 128
                    nc.vector.tensor_copy(
                        v_sb[:, 128 * vch0:128 * vch0 + HNW], tr[:])

            def qkv_hn(hn):
                """QKV projection for one 512-token block; m-outer, yields
                after each matmul so attention chunks can interleave."""
                b = hn // (HN // B)
                s0 = (hn % (HN // B)) * HNW   # within-batch token offset
                xt_t = xt_tiles[hn]
                for m in range(MCH):
                    ps = bigp[0].tile([128, 512], F32, tag="big",
                                      name=f"qkv{hn}_{m}")
                    for k in range(KCH):
                        c0 = (k * MCH + m) * 128
                        nc.tensor.matmul(
                            ps[:], wq_sb[:, c0:c0 + 128],
                            xt_t[:, k * HNW:(k + 1) * HNW],
                            start=(k == 0), stop=(k == KCH - 1))
                        yield
                    yield from evac_m(hn, b, s0, m, ps, bigp[0])

            def qkv_hn0_kmajor(ps0):
                """Stage-0 QKV for block 0: k-outer over 6 concurrent PSUM
                groups while the DMA stream is the limiter (k < KT), then
                finish each group m-sequentially so evacs+ropes stagger."""
                KT = KCH - 8
                xt_t = xt_tiles[0]
                # dummy matmuls on ident while the first DMAs land: starts
                # the PE p-state (HAM) ramp ~3us early, so the real matmuls
                # reach full clock sooner
                warm = ps0.tile([128, 128], F32, tag="warm", name="warm")
                for _ in range(16):
                    nc.tensor.matmul(warm[:], ident[:], ident[:],
                                     start=True, stop=True)
                groups = [ps0.tile([128, 512], F32, tag=f"q{m}",
                                   name=f"qkv0_{m}") for m in range(MCH)]
                for k in range(KT):
                    for m in range(MCH):
                        c0 = (k * MCH + m) * 128
                        nc.tensor.matmul(
                            groups[m][:], wq_sb[:, c0:c0 + 128],
                            xt_t[:, k * HNW:(k + 1) * HNW],
                            start=(k == 0), stop=False)
                for m in (GPC, 0, 1, 2, 3, MCH - 1):  # k, q0-3, v
                    for k in range(KT, KCH):
                        c0 = (k * MCH + m) * 128
                        nc.tensor.matmul(
                            groups[m][:], wq_sb[:, c0:c0 + 128],
                            xt_t[:, k * HNW:(k + 1) * HNW],
                            start=False, stop=(k == KCH - 1))
                    for _ in evac_m(0, 0, 0, m, groups[m], ps0):
                        pass

            def load_wo(hb):
                wo_t = wop.tile([128, GPC * HBW], F16, tag="wo",
                                name=f"wo{hb}")
                c0 = hb * GPC * HBW
                nc.sync.dma_start(wo_t[:], wo[:, c0:c0 + GPC * HBW])
                return wo_t

            def oproj_tail(sc_list, preloaded):
                """Tail o_proj pass (b=1): the first 4 groups emit their
                g<3 matmuls up front (they only need earlier finalizes) so
                the PE stays busy through the last block's finalize chain;
                alternates the op/big PSUM rings for depth 4."""
                b = 1
                engs = [nc.scalar, nc.vector]
                wo_tiles = dict(preloaded)

                def ensure(hb):
                    if hb < NHB and hb not in wo_tiles:
                        wo_tiles[hb] = load_wo(hb)

                ensure(0)
                ensure(1)
                units = []
                for hbp in range(NHB // 2):
                    for sc in sc_list:
                        for h2 in range(2):
                            units.append((2 * hbp + h2, sc, hbp, h2))
                NPRO = 4
                pro_ps = []
                for i, (hb, sc, hbp, h2) in enumerate(units[:NPRO]):
                    pool = opp[0] if i % 2 == 0 else bigp[0]
                    ops = pool.tile([128, HBW], F32,
                                    tag="op" if i % 2 == 0 else "big",
                                    name=f"opt{sc}_{hb}")
                    pro_ps.append(ops)
                    for g in range(GPC - 1):
                        lcol = b * GPC * S + g * S + 128 * sc
                        nc.tensor.matmul(
                            ops[:], outT_sb[:, lcol:lcol + 128],
                            wo_tiles[hb][:, g * HBW:(g + 1) * HBW],
                            start=(g == 0), stop=False)
                ys = None
                for i, (hb, sc, hbp, h2) in enumerate(units):
                    if i == 2:
                        ensure(2)
                        ensure(3)
                    elif i == 10:
                        ensure(4)
                        ensure(5)
                    elif i == 18:
                        ensure(6)
                        ensure(7)
                    eng = engs[i % 2]
                    if i < NPRO:
                        ops = pro_ps[i]
                        g0 = GPC - 1
                    else:
                        pool = opp[0] if i % 2 == 0 else bigp[0]
                        ops = pool.tile([128, HBW], F32,
                                        tag="op" if i % 2 == 0 else "big",
                                        name=f"opt{sc}_{hb}")
                        g0 = 0
                    for g in range(g0, GPC):
                        lcol = b * GPC * S + g * S + 128 * sc
                        nc.tensor.matmul(
                            ops[:], outT_sb[:, lcol:lcol + 128],
                            wo_tiles[hb][:, g * HBW:(g + 1) * HBW],
                            start=(g == 0 and g0 == 0),
                            stop=(g == GPC - 1))
                    if h2 == 0:
                        ys = ysp.tile([128, 2 * HBW], F16, tag="ys",
                                      name=f"yst{sc}_{hbp}")
                    dst = ys[:, h2 * HBW:(h2 + 1) * HBW]
                    if eng is nc.scalar:
                        eng.copy(dst, ops[:])
                    else:
                        eng.tensor_copy(dst, ops[:])
                    nc.sync.dma_start(
                        y[b * S + 128 * sc:b * S + 128 * (sc + 1),
                          HBW * hb:HBW * (hb + 1)], dst)

            def oproj_pass(b, sc_list, preloaded=None, single_dma=False,
                           tail_pre=None):
                """One o_proj pass: hb-pair-outer; per group 4 contraction
                matmuls + evac copy into half a [128,1024] ys tile; one y DMA
                per pair. wo tiles roll through `wop` (SWDGE loads) with a
                one-pair prefetch distance."""
                engs = [nc.scalar, nc.vector]
                i = 0
                wo_tiles = dict(preloaded or {})

                def ensure(hb):
                    if hb < NHB and hb not in wo_tiles:
                        wo_tiles[hb] = load_wo(hb)

                ensure(0)
                ensure(1)
                for hbp in range(NHB // 2):
                    for si, sc in enumerate(sc_list):
                        for h2 in range(2):
                            hb = 2 * hbp + h2
                            eng = engs[i % 2]
                            i += 1

                            def unit(b=b, sc=sc, hb=hb, h2=h2, si=si,
                                     hbp=hbp, eng=eng, wo_t=wo_tiles[hb]):
                                if si == 0 and h2 == 0:
                                    ensure(2 * hbp + 2)
                                    ensure(2 * hbp + 3)
                                    if tail_pre is not None \
                                            and hbp == NHB // 2 - 1:
                                        tail_pre[0] = load_wo(0)
                                        tail_pre[1] = load_wo(1)
                                ops = opp[0].tile([128, HBW], F32, tag="op",
                                                  name=f"op{b}_{sc}_{hb}")
                                for g in range(GPC):
                                    lcol = b * GPC * S + g * S + 128 * sc
                                    nc.tensor.matmul(
                                        ops[:], outT_sb[:, lcol:lcol + 128],
                                        wo_t[:, g * HBW:(g + 1) * HBW],
                                        start=(g == 0), stop=(g == GPC - 1))
                                if h2 == 0:
                                    ys = ysp.tile([128, 2 * HBW], F16,
                                                  tag="ys",
                                                  name=f"ys{b}_{sc}_{hbp}")
                                    oproj_pass.ys = ys
                                ys = oproj_pass.ys
                                dst = ys[:, h2 * HBW:(h2 + 1) * HBW]
                                if eng is nc.scalar:
                                    eng.copy(dst, ops[:])
                                else:
                                    eng.tensor_copy(dst, ops[:])
                                if single_dma:
                                    nc.sync.dma_start(
                                        y[b * S + 128 * sc:
                                          b * S + 128 * (sc + 1),
                                          HBW * hb:HBW * (hb + 1)], dst)
                                elif h2 == 1:
                                    nc.sync.dma_start(
                                        y[b * S + 128 * sc:
                                          b * S + 128 * (sc + 1),
                                          1024 * hbp:1024 * (hbp + 1)],
                                        ys[:])

                            yield unit

            # finalize: normalize one attention block's output.
            # Split in two so PE fillers sit between the sums matmul and
            # the broadcast matmul (which waits on the DVE reciprocal).
            def finalize_a(pend):
                f_acc, f_ot, f_ocol = pend
                sums = psp.tile([128, 512], F32, tag="sc", name="sums")
                nc.tensor.matmul(sums[0:1, :], ones[:], f_acc[:],
                                 start=True, stop=True)
                rc = rcp.tile([1, 512], F16, tag="rc", name="rc")
                with nc.allow_low_precision(reason="softmax denom fits fp16"):
                    nc.vector.reciprocal(rc[:], sums[0:1, :])
                return (rc, f_ot, f_ocol)

            def finalize_b(pend2):
                rc, f_ot, f_ocol = pend2
                bc = bcp.tile([128, 512], F16, tag="bc", name="bc")
                nc.gpsimd.partition_broadcast(bc[:], rc[:])
                nc.vector.tensor_mul(outT_sb[:, f_ocol:f_ocol + 512],
                                     f_ot[:], bc[:])

            pending = [None]

            def attn_block(b, g, j, fillers, cadence):
                """One attention s-block (512 queries): scores+exp+pv over
                n_t kv chunks, pipelined; pulls `cadence` (fractional) filler
                units from `fillers` after each chunk's scores matmul.
                Boundary chunks only compute the visible query subrange."""
                scol = g * BS + b * S + j * 512
                n_t = (P // 128) + 4 * (j + 1)      # causal skip
                acc = accp.tile([128, 512], F16, tag="acc",
                                name=f"acc{b}{g}{j}")
                ot_ps = psp.tile([128, 512], F32, tag="ot",
                                 name=f"ot{b}{g}{j}")
                prev = None
                credit = 0.0
                for ti in range(n_t):
                    if ti < 8:
                        kcol = b * T + 128 * ti
                    else:
                        kcol = b * T + P + 128 * (ti - 8)
                    vch = 16 * b + ti
                    r_idx = (ti - 8) - 4 * j
                    s_lo = 128 * r_idx if (ti >= 8 and 0 <= r_idx < 4) else 0
                    sc_ps = psp.tile([128, 512], F32, tag="sc", name="sc")
                    nc.tensor.matmul(sc_ps[:, s_lo:512],
                                     kT[:, kcol:kcol + 128],
                                     qT[:, scol + s_lo:scol + 512],
                                     start=True, stop=True)
                    pt = ptp.tile([128, 512], F16, tag="pt", name="pt")
                    nc.scalar.activation(pt[:, s_lo:512], sc_ps[:, s_lo:512],
                                         mybir.ActivationFunctionType.Exp,
                                         scale=SCALE)
                    if ti >= 8 and 0 <= r_idx < 4:
                        # diagonal 128-col strip: in-chunk causal triangle
                        nc.vector.tensor_mul(
                            pt[:, s_lo:s_lo + 128], pt[:, s_lo:s_lo + 128],
                            mask_sb[:])
                    if ti == 0:
                        nc.vector.tensor_copy(acc[:], pt[:])
                    else:
                        nc.vector.tensor_add(acc[:, s_lo:512],
                                             pt[:, s_lo:512],
                                             acc[:, s_lo:512])
                    # fillers between the scores and the previous pv;
                    # trimmed chunks leave more PE idle, so weight them up
                    credit += cadence + 3.0 * (s_lo / 512.0)
                    while credit >= 1.0:
                        credit -= 1.0
                        if not next_filler(fillers):
                            break
                    if prev is not None:
                        p_pt, p_vch, p_slo, p_first = prev
                        nc.tensor.matmul(
                            ot_ps[:, p_slo:512],
                            v_sb[:, 128 * p_vch:128 * (p_vch + 1)],
                            p_pt[:, p_slo:512], start=p_first, stop=False)
                    prev = (pt, vch, s_lo, ti == 0)
                    if ti == 0 and pending[0] is not None:
                        attn_block.pend2 = finalize_a(pending[0])
                        pending[0] = None
                    elif ti == 4 and attn_block.pend2 is not None:
                        finalize_b(attn_block.pend2)
                        attn_block.pend2 = None
                p_pt, p_vch, p_slo, p_first = prev
                nc.tensor.matmul(ot_ps[:, p_slo:512],
                                 v_sb[:, 128 * p_vch:128 * (p_vch + 1)],
                                 p_pt[:, p_slo:512], start=p_first, stop=True)
                ocol = b * GPC * S + g * S + j * 512
                pending[0] = (acc, ot_ps, ocol)

            attn_block.pend2 = None

            def next_filler(fillers):
                while fillers:
                    try:
                        u = next(fillers[0])
                        if callable(u):
                            u()
                        return True
                    except StopIteration:
                        fillers.pop(0)
                return False

            def drain(fillers):
                while next_filler(fillers):
                    pass

            # ---- schedule -------------------------------------------------
            # stage 0: qkv(hn0) k-major in its own 6-bank+tr PSUM scope
            with tc.tile_pool(name="ps0", bufs=1, space="PSUM") as ps0:
                qkv_hn0_kmajor(ps0)
            # Gate the xt2 DMA pieces on hn0's last rope: its half-swap DMAs
            # share the DMA engines with xt2's 4MB, and losing that race
            # stalls stage-1 attention. Tiny copies (rope-dependent) into the
            # xt slot xt2 will reuse push xt2 behind the swaps.
            m3c = 3 * BS
            xt0_t = xt_tiles[0]
            for i in range(8):
                nc.vector.tensor_copy(
                    xt0_t[0:1, i * (KCH * HNW // 8):i * (KCH * HNW // 8) + 1],
                    qT[0:1, m3c:m3c + 1])
            bigp = [None]
            opp = [None]
            with tc.tile_pool(name="ps", bufs=2, space="PSUM") as psp:
                with tc.tile_pool(name="psbig12", bufs=2,
                                  space="PSUM") as big12:
                    bigp[0] = big12
                    # stage 1: attn(b0, j=0) + qkv(hn1). Pre-pull ~32 units
                    # so the PE has work while hn0's rope chains complete.
                    fill = [qkv_hn(1)]
                    for _ in range(32):
                        next_filler(fill)
                    for g in range(GPC):
                        attn_block(0, g, 0, fill, cadence=3.2)
                        if g == 2:
                            xt_load(2, split=8)
                    drain(fill)
                    # stage 2: attn(b0, j=1) + qkv(hn2)
                    fill = [qkv_hn(2)]
                    for g in range(GPC):
                        attn_block(0, g, 1, fill, cadence=2.85)
                        if g == 1:
                            xt_load(3, split=8)
                        elif g == 2:
                            kv_load(1)
                    drain(fill)
                # stage 3: attn(b1, j=0) + qkv(hn3) + o_proj pass A (b0)
                big34 = tc.alloc_tile_pool(name="psbig", bufs=2,
                                           space="PSUM")
                op34 = tc.alloc_tile_pool(name="psop", bufs=2, space="PSUM")
                bigp[0] = big34
                opp[0] = op34
                gen_qkv3 = qkv_hn(3)
                genA = oproj_pass(0, range(8))
                fill = [gen_qkv3, genA]
                for g in range(GPC):
                    attn_block(1, g, 0, fill, cadence=4.1)
                drain([gen_qkv3])
                # stage 4: attn(b1, j=1) + o_proj passes A remainder + B.
                # Prime pass B so its first wo tile loads ahead of use.
                tail_pre = {}
                genB = oproj_pass(1, range(0, 4), tail_pre=tail_pre)
                fill = [genA, genB]
                for g in range(GPC):
                    attn_block(1, g, 1, fill, cadence=1.2)
                # normalize the last block while leftover fillers keep PE
                # busy, then the b1/j1-dependent o_proj tail
                # final block: half-width finalize chain so the tail's
                # first g3 matmuls (sc 4-5) unblock earlier
                f_acc, f_ot, f_ocol = pending[0]
                pending[0] = None
                sums = psp.tile([128, 512], F32, tag="sc", name="sums_l")
                nc.tensor.matmul(sums[0:1, :], ones[:], f_acc[:],
                                 start=True, stop=True)
                rc = rcp.tile([1, 512], F16, tag="rc", name="rc_l")
                halves = []
                with nc.allow_low_precision(reason="softmax denom fp16"):
                    for hf in range(2):
                        sl = slice(256 * hf, 256 * (hf + 1))
                        nc.vector.reciprocal(rc[0:1, sl], sums[0:1, sl])
                        bc = bcp.tile([128, 256], F16, tag="bch",
                                      name=f"bch{hf}")
                        nc.gpsimd.partition_broadcast(bc[:], rc[0:1, sl])
                        halves.append((bc, sl))
                drain(fill)
                for bc, sl in halves:
                    nc.vector.tensor_mul(
                        outT_sb[:, f_ocol + sl.start:f_ocol + sl.stop],
                        f_ot[:, sl], bc[:])
                oproj_tail(range(4, 8), tail_pre)
                op34.release()
                big34.release()

    nc.compile()
    return nc


_PROGRAM = None


def _get_program():
    global _PROGRAM
    if _PROGRAM is None:
        _PROGRAM = _build_program()
    return _PROGRAM


def _shard_inputs(hidden_states, w_qkv, w_o, cos, sin, k_cache, v_cache):
    """Build the 8 per-core input maps (numpy, fp16)."""
    hs = np.asarray(hidden_states, np.float32)
    w_qkv = np.asarray(w_qkv, np.float32)
    w_o = np.asarray(w_o, np.float32)
    cos = np.asarray(cos, np.float32)
    sin = np.asarray(sin, np.float32)
    k_cache = np.asarray(k_cache, np.float32)
    v_cache = np.asarray(v_cache, np.float32)

    xT = np.ascontiguousarray(hs.reshape(BS, H).T.astype(np.float16))
    cosT = np.ascontiguousarray(cos.T.astype(np.float16))
    ssinT = sin.T.astype(np.float16).copy()
    ssinT[0:64] *= -1.0
    ssinT = np.ascontiguousarray(ssinT)

    # lower-triangle tile: mask[t, s] = (s >= t)
    tl = np.arange(128)[:, None]
    sl = np.arange(128)[None, :]
    mask = np.ascontiguousarray((sl >= tl).astype(np.float16))

    in_maps = []
    for c in range(NCORES):
        wq_c = w_qkv[:, c * GPC * D:(c + 1) * GPC * D]
        wk_c = w_qkv[:, NQ * D + c * D:NQ * D + (c + 1) * D]
        wv_c = w_qkv[:, (NQ + NKV) * D + c * D:(NQ + NKV) * D + (c + 1) * D]
        wc = np.concatenate([wq_c, wk_c, wv_c], axis=1)      # [H, 768]
        # k-major: [p, k*MCH*128 + m*128 + col]
        wqkv_r = np.ascontiguousarray(
            wc.reshape(KCH, 128, MCH, 128).transpose(1, 0, 2, 3)
            .reshape(128, KCH * MCH * 128).astype(np.float16))
        wo_c = w_o[c * GPC * D:(c + 1) * GPC * D, :]          # [512, H]
        # hb-major: [p, hb*GPC*512 + g*512 + col]
        wo_r = np.ascontiguousarray(
            wo_c.reshape(GPC, 128, NHB, HBW).transpose(1, 2, 0, 3)
            .reshape(128, NHB * GPC * HBW).astype(np.float16))
        kcT = np.ascontiguousarray(
            k_cache[:, :, c, :].reshape(B * P, D).T.astype(np.float16))
        vc = np.ascontiguousarray(
            v_cache[:, :, c, :].reshape(B * P, D).astype(np.float16))
        in_maps.append(dict(xT=xT, wqkv=wqkv_r, wo=wo_r, cosT=cosT,
                            ssinT=ssinT, kcT=kcT, vc=vc, masks=mask))
    return in_maps


def _run(in_maps, trace=False):
    nc = _get_program()
    return run_bass_kernel_spmd(nc, in_maps, list(range(NCORES)), trace=trace)


def kernel(hidden_states, w_qkv, w_o, cos, sin, k_cache, v_cache):
    in_maps = _shard_inputs(hidden_states, w_qkv, w_o, cos, sin,
                            k_cache, v_cache)
    res = _run(in_maps)
    acc = np.zeros((BS, H), np.float64)
    for c in range(NCORES):
        acc += res.results[c]["y"]
    return acc.astype(np.float32).reshape(B, S, H)
